# revision 34
# baseline (speedup 1.0000x reference)
"""Trainium2 Bass kernel for nn_AdaptiveAngleConv.

Reference computes, for each of 5 angles, a bilinear "deformable" 3x3
sampling of x (2,256,64,64) into a (2,256,192,192) image, then a 3x3
VALID conv (stride 1) with a shared weight (256,256,3,3), giving 5
outputs of (2,256,190,190).

Key math: the reference's clipped bilinear sampling is exactly an
UNclipped separable 2x2 stencil with constant per-(angle, n)
coefficients on a zero-padded x — every clipped index lands on a
zero-pad row/col, so the clip never changes a nonzero contribution.
Angles 0/90/180 have integer offsets (pure shifted copies); 45/135 need
a 2-pass (rows then cols) lerp.

Sharding: output rows are split across the 8 cores (24 rows each).
Each core receives a pre-sliced 13-row input slab so the SPMD graph is
identical on every core; no collectives.

The conv itself is 1D row-Winograd to cut PE work below the 9
MACs/output-pixel of direct 3x3 conv:
  - angles 90/180: F(4,3)  -> 4.5 MACs/pixel. The sampled image xo is
    built with strided copies on the Scalar engine; the Winograd input
    transform (V = BT @ xo row-windows) runs on Vector with the 2-input
    temps on GpSimd; matmuls accumulate 6 M_u chains per 4-row group in
    PSUM; Scalar evacuates PSUM->SBUF as fp16; Vector applies AT to get
    the 4 output rows.
  - angles 45/135: F(2,3)  -> 6 MACs/pixel (their lerp-based xo build
    needs Vector time, so the cheaper-transform variant keeps Vector
    under the PE window).
  - angle 0: phase-collapsed direct conv (49 taps per 3x3 phase block vs
    81) reading the input slab directly — no sampled image at all.
Outputs are written to DRAM in fp16 (host upcasts); the added ~5e-4
relative error is far inside the 2e-2 gate.

The two angle-0 jobs bracket the schedule (first for batch 0, last for
batch 1) to minimize pipeline head/tail; fp16 keeps the PE at 1
cycle/row with ~8x better rounding than bf16.
"""

import os
import sys

for _p in ("/opt/trn_rl_repo", "/root/.axon_site/_ro/trn_rl_repo"):
    if os.path.isdir(_p) and _p not in sys.path:
        sys.path.insert(0, _p)

import numpy as np

import concourse.bass as bass
import concourse.mybir as mybir
from concourse import bacc, tile
from concourse.alu_op_type import AluOpType
from concourse.bass_utils import run_bass_kernel_spmd

F32 = mybir.dt.float32
F16 = mybir.dt.float16

S2 = 2 ** 0.5
ANGLES = [0, 45, 90, 135, 180]
_OFF = {
    0: ([0.0] * 9, [0.0] * 9),
    1: ([1 - S2, 1 - S2 * 0.5, 1, -S2 * 0.5, 0, S2 * 0.5, -1, S2 * 0.5 - 1, S2 - 1],
        [1, S2 * 0.5, S2 - 1, 1 - S2 * 0.5, 0, S2 * 0.5 - 1, 1 - S2, -S2 * 0.5, -1]),
    2: ([0, 1, 2, -1, 0, 1, -2, -1, 0],
        [2, 1, 0, 1, 0, -1, 0, -1, -2]),
    3: ([1, 1 + S2 * 0.5, 1 + S2, -S2 * 0.5, 0, S2 * 0.5, -1 - S2, -1 - S2 * 0.5, -1],
        [1 + S2, S2 * 0.5, -1, 1 + S2 * 0.5, 0, -1 - S2 * 0.5, 1, -S2 * 0.5, 1 + S2]),
    4: ([2, 2, 2, 0, 0, 0, -2, -2, -2],
        [2, 0, -2, 2, 0, -2, 2, 0, -2]),
}

NCORES = 8
NR = 24            # output rows per core (8*24 = 192, rows 190/191 dropped)
SLAB_ROWS = 13     # input rows a core needs: hi in [8k-2, 8k+10]
SLAB_COLS = 70     # data cols -2..67
XO_ROWS = 26       # NR + 2 halo rows of the sampled image
XO_F = XO_ROWS * 192
XO_ROWS2 = 28      # xo tile rows incl. 2 pad rows (spanned, never read, by
                   # the rows-of-4 rearrange views in wino_job_f4)
XO2F = XO_ROWS2 * 192

# Winograd F(m,3) matrices (Lavin).  BT/AT are encoded directly as the op
# sequences below; G is used on the host for the weight transform.
G43 = np.array([
    [1 / 4, 0, 0],
    [-1 / 6, -1 / 6, -1 / 6],
    [-1 / 6, 1 / 6, -1 / 6],
    [1 / 24, 1 / 12, 1 / 6],
    [1 / 24, -1 / 12, 1 / 6],
    [0, 0, 1]], dtype=np.float32)
G23 = np.array([
    [1, 0, 0],
    [0.5, 0.5, 0.5],
    [0.5, -0.5, 0.5],
    [0, 0, 1]], dtype=np.float32)

WINO_M = {45: 2, 135: 2, 90: 4, 180: 4}   # F(m,3) per angle


def _tables():
    """Per angle: list of (n, r, s, Ax, fx, Ay, fy) in f32 semantics."""
    rng = np.arange(-1, 2)
    pnx, pny = np.meshgrid(rng, rng, indexing="ij")
    pnx = pnx.reshape(-1).astype(np.float32)
    pny = pny.reshape(-1).astype(np.float32)
    out = {}
    for a in ANGLES:
        ox, oy = _OFF[a // 45]
        dx = pnx + np.array(ox, dtype=np.float32)
        dy = pny + np.array(oy, dtype=np.float32)
        rows = []
        for n in range(9):
            Ax = int(np.floor(dx[n]))
            Ay = int(np.floor(dy[n]))
            fx = float(np.float32(dx[n] - Ax))
            fy = float(np.float32(dy[n] - Ay))
            rows.append((n, n // 3, n % 3, Ax, fx, Ay, fy))
        out[a] = rows
    return out


TABLES = _tables()
# distinct fractional row offsets shared by the 45/135 pair
LERP_DS = sorted({(t[3], t[4]) for a in (45, 135) for t in TABLES[a]})

# Angle-0 phase-collapsed conv: output phase rho uses row taps di with the
# listed combo of original kernel rows (g(m)=m//3+m%3-1 collides for m=1,3
# and m=2,4). Combo indices into the host-precomputed sums: 0,1,2 = single
# ki, 3 = ki0+ki2. Same structure for columns. 49 taps/phase-grid vs 81.
ROW_COMBOS = [(0,), (1,), (2,), (0, 2)]
PHROWS = {0: [(-1, 0), (0, 1), (1, 2)],
          1: [(0, 3), (1, 1)],
          2: [(1, 3), (0, 1)]}


def build_graph():
    nc = bacc.Bacc()
    xs = nc.declare_dram_parameter("xs", [2, 2, 128, SLAB_ROWS * SLAB_COLS], F16, False)
    ww = nc.declare_dram_parameter("ww", [2, 128, 6 * 3 * 2 * 128], F16, False)
    w2 = nc.declare_dram_parameter("w2", [2, 128, 4 * 3 * 2 * 128], F16, False)
    wc = nc.declare_dram_parameter("wc", [2, 128, 16 * 2 * 128], F16, False)
    out = nc.declare_dram_parameter("out", [5, 2, 2, 128, NR, 190], F16, True)

    with tile.TileContext(nc) as tc:
        with (
            tc.tile_pool(name="const", bufs=1) as constp,
            tc.tile_pool(name="xop", bufs=2) as xop,
            tc.tile_pool(name="rcp", bufs=1) as rcp,
            tc.tile_pool(name="vp", bufs=2) as vp,
            tc.tile_pool(name="vtmp", bufs=1) as vtp,
            tc.tile_pool(name="mp", bufs=2) as mpp,
            tc.tile_pool(name="stg", bufs=2) as stgp,
            tc.tile_pool(name="ps", bufs=2, space="PSUM") as psp,
        ):
            # HAM warm-up: dependency-free matmuls on an uninitialized tile
            # keep the PE busy during the input-DMA window so the clock gate
            # is already at 8/8 when the first real matmul issues.
            warm = constp.tile([128, 384], F16, name="warm", tag="warm")
            nc.gpsimd.memset(warm[:], 0.0)
            wps = psp.tile([128, 1536], F32, name="wps", tag="ps")
            for _ in range(16):
                nc.tensor.matmul(wps[:, :256], warm[:, :128], warm[:, 128:384],
                                 start=True, stop=True)

            # DMA order matters for the head: the first job (collapsed
            # angle-0, batch 0) needs slab b0 + wc only; w2 is needed one
            # job later, ww three jobs later.
            slab = {}

            def load_slab(b):
                for ct in range(2):
                    s = constp.tile([128, SLAB_ROWS * SLAB_COLS], F16,
                                    name=f"slab{b}{ct}", tag=f"slab{b}{ct}")
                    nc.sync.dma_start(s[:], xs[b, ct])
                    slab[(b, ct)] = s

            load_slab(0)
            # wc is ot-major; load the ot=0 half first so the first job's
            # first matmuls only wait on half the collapsed-weight bytes.
            wc_sb = []
            for ct in range(2):
                wctile = constp.tile([128, 16 * 2 * 128], F16, name=f"wc{ct}",
                                     tag=f"wc{ct}")
                nc.sync.dma_start(wctile[:, :2048], wc[ct][:, :2048])
                wc_sb.append(wctile)
            for ct in range(2):
                nc.sync.dma_start(wc_sb[ct][:, 2048:], wc[ct][:, 2048:])
            w2_sb = []
            for ct in range(2):
                w2t = constp.tile([128, 4 * 3 * 2 * 128], F16, name=f"w2{ct}",
                                  tag=f"w2{ct}")
                nc.sync.dma_start(w2t[:], w2[ct])
                w2_sb.append(w2t)
            load_slab(1)
            ww_sb = []
            for ct in range(2):
                wwt = constp.tile([128, 6 * 3 * 2 * 128], F16, name=f"ww{ct}",
                                  tag=f"ww{ct}")
                nc.sync.dma_start(wwt[:], ww[ct])
                ww_sb.append(wwt)

            def slab3(b, ct):
                return slab[(b, ct)].rearrange("p (r c) -> p r c", c=SLAB_COLS)

            def build_xo_int(a, b):
                """xo tiles for an integer-offset angle via strided copies
                on the Scalar engine (Vector is the scarce resource)."""
                xo = []
                for ct in range(2):
                    t = xop.tile([128, XO2F], F16, name=f"xo{ct}", tag=f"xo{ct}")
                    v = t[:, :XO_F].rearrange("p (r c) -> p r c", c=192)
                    sv = slab3(b, ct)
                    for (n, r, s, Ax, fx, Ay, fy) in TABLES[a]:
                        nrow = 9 if r < 2 else 8
                        src = sv[:, 2 + Ax : 2 + Ax + nrow, 2 + Ay : 66 + Ay]
                        nc.scalar.copy(v[:, r::3, s::3], src)
                    xo.append(t)
                return xo

            # (Ax, fx) row-offsets that some fy!=0 tap reads: only these
            # need a col-diff C tile.
            needs_c = {(t[3], t[4]) for a in (45, 135) for t in TABLES[a]
                       if t[6] != 0.0}

            def build_lerp_rc(b):
                """Shared row-lerp R_d and col-diff C_d tiles for 45+135."""
                R = {}
                C = {}
                for ct in range(2):
                    sv = slab3(b, ct)
                    dr = rcp.tile([128, 12 * SLAB_COLS], F16,
                                  name=f"dr{ct}", tag="dr")
                    drv = dr.rearrange("p (r c) -> p r c", c=SLAB_COLS)
                    nc.vector.tensor_tensor(drv, sv[:, 1:13, :], sv[:, 0:12, :],
                                            AluOpType.subtract)
                    for di, (Ax, fx) in enumerate(LERP_DS):
                        if fx == 0.0:
                            rv = sv[:, 2 + Ax : 11 + Ax, :]
                        else:
                            rt = rcp.tile([128, 9 * SLAB_COLS], F16,
                                          name=f"r{ct}_{di}", tag=f"r{ct}_{di}")
                            rv = rt.rearrange("p (r c) -> p r c", c=SLAB_COLS)
                            nc.vector.scalar_tensor_tensor(
                                rv, drv[:, 2 + Ax : 11 + Ax, :], fx,
                                sv[:, 2 + Ax : 11 + Ax, :],
                                AluOpType.mult, AluOpType.add)
                        R[(ct, Ax, fx)] = rv
                        if (Ax, fx) in needs_c:
                            ctile = rcp.tile([128, 9 * SLAB_COLS], F16,
                                             name=f"c{ct}_{di}", tag=f"c{ct}_{di}")
                            cv = ctile.rearrange("p (r c) -> p r c", c=SLAB_COLS)
                            nc.vector.tensor_tensor(cv[:, :, 0:69], rv[:, :, 1:70],
                                                    rv[:, :, 0:69], AluOpType.subtract)
                            C[(ct, Ax, fx)] = cv
                return R, C

            def build_xo_lerp(a, b, R, C):
                xo = []
                for ct in range(2):
                    t = xop.tile([128, XO2F], F16, name=f"xo{ct}", tag=f"xo{ct}")
                    v = t[:, :XO_F].rearrange("p (r c) -> p r c", c=192)
                    for (n, r, s, Ax, fx, Ay, fy) in TABLES[a]:
                        nrow = 9 if r < 2 else 8
                        rv = R[(ct, Ax, fx)][:, :nrow, 2 + Ay : 66 + Ay]
                        if fy == 0.0:
                            nc.scalar.copy(v[:, r::3, s::3], rv)
                        else:
                            cv = C[(ct, Ax, fx)][:, :nrow, 2 + Ay : 66 + Ay]
                            nc.vector.scalar_tensor_tensor(
                                v[:, r::3, s::3], cv, fy, rv,
                                AluOpType.mult, AluOpType.add)
                    xo.append(t)
                return xo

            def conv_job_angle0(ai, b):
                """Phase-collapsed conv reading the slab directly (no xo).
                Groups by output row phase rho so each rho's rows can DMA
                out as soon as its three column phases are evacuated."""
                for ot in range(2):
                    for rho in range(3):
                        stg = stgp.tile([128, 8 * 192], F16, name="stg", tag="stg")
                        stgv = stg.rearrange("p (r c) -> p r c", c=192)
                        ps = psp.tile([128, 1536], F32, name="ps", tag="ps")
                        for sig in range(3):
                            taps = [(di, dj, ri * 4 + ci)
                                    for (di, ri) in PHROWS[rho]
                                    for (dj, ci) in PHROWS[sig]]
                            nmm = len(taps) * 2
                            i = 0
                            for (di, dj, cb) in taps:
                                for ct in range(2):
                                    sv = slab3(b, ct)
                                    w_ap = wc_sb[ct][:, (ot * 16 + cb) * 128 :
                                                     (ot * 16 + cb + 1) * 128]
                                    nc.tensor.matmul(
                                        ps[:, sig * 512 : (sig + 1) * 512],
                                        w_ap,
                                        sv[:, di + 2 : di + 10, dj + 2 : dj + 66],
                                        start=(i == 0), stop=(i == nmm - 1))
                                    i += 1
                        for sig in range(3):
                            psv = ps[:, sig * 512 : (sig + 1) * 512].rearrange(
                                "p (r c) -> p r c", c=64)
                            nc.scalar.copy(stgv[:, :, sig::3], psv)
                        nc.sync.dma_start(out[ai, b, ot, :, rho::3, :],
                                          stgv[:, :, :190])

            def wino_job_f4(ai, b, xo):
                """F(4,3) 1D row-Winograd conv of xo -> 24x190 outputs.
                6 groups of 4 output rows; per group 6 M_u chains of
                3(kj) x 2(ct) accumulating matmuls; AT combine on Vector.

                scalar_tensor_tensor runs 1x on the DVE, so every scaled
                combine is split into tensor_scalar (4x) + tensor_tensor
                (2x), and adjacent-row operand pairs are fused into one
                [128,3,2,192] op via a rows-of-4 rearrange of xo."""
                xov = [xo[ct][:, :XO_ROWS2 * 192].rearrange("p (r c) -> p r c", c=192)
                       for ct in range(2)]
                for half in range(2):
                    g0 = 3 * half

                    def dpairc(ct, a):
                        """rows (4g+a, 4g+a+1) for the half's 3 groups:
                        [128, 3, 2, 192]."""
                        a0 = 4 * g0 + a
                        v4 = xov[ct][:, a0 : a0 + 12, :].rearrange(
                            "p (g r) c -> p g r c", r=4)
                        return v4[:, :, 0:2, :]

                    def dpair2(ct, a):
                        """rows (4g+a, 4g+a+2): [128, 3, 2, 192]."""
                        a0 = 4 * g0 + a
                        v4 = xov[ct][:, a0 : a0 + 12, :].rearrange(
                            "p (g r) c -> p g r c", r=4)
                        return v4[:, :, 0:3:2, :]

                    vt = []
                    for ct in range(2):
                        t = vp.tile([128, 3 * 6 * 192], F16, name=f"v{ct}",
                                    tag=f"v{ct}")
                        v = t.rearrange("p (g u c) -> p g u c", u=6, c=192)
                        tmp = vtp.tile([128, 3 * 8 * 192], F16, name=f"vt{ct}",
                                       tag=f"vt{ct}")
                        tm = tmp.rearrange("p (g i c) -> p g i c", i=8, c=192)
                        pairc = lambda i: tm[:, :, i : i + 2, :]
                        # ab = -5*(d2,d3); tAtB = ab + (d4,d5); ce = 4*(d0,d1)
                        nc.vector.tensor_scalar_mul(pairc(0), dpairc(ct, 2), -5.0)
                        nc.vector.tensor_tensor(pairc(2), pairc(0), dpairc(ct, 4),
                                                AluOpType.add)
                        nc.vector.tensor_scalar_mul(pairc(4), dpairc(ct, 0), 4.0)
                        # (v0, v5) = ce + tAtB
                        nc.vector.tensor_tensor(v[:, :, 0:6:5, :],
                                                tm[:, :, 4:6, :], tm[:, :, 2:4, :],
                                                AluOpType.add)
                        # s = (d1,d3)+(d2,d4); m = (d1,d3)-(d2,d4)  [GpSimd]
                        nc.gpsimd.tensor_tensor(tm[:, :, 6:8, :], dpair2(ct, 1),
                                                dpair2(ct, 2), AluOpType.add)
                        nc.gpsimd.tensor_tensor(tm[:, :, 0:2, :], dpair2(ct, 1),
                                                dpair2(ct, 2), AluOpType.subtract)
                        # v1 = -4*s1 + s2 ; v2 = 4*m1 - m2
                        nc.vector.tensor_scalar_mul(tm[:, :, 2], tm[:, :, 6], -4.0)
                        nc.vector.tensor_tensor(v[:, :, 1], tm[:, :, 2], tm[:, :, 7],
                                                AluOpType.add)
                        nc.vector.tensor_scalar_mul(tm[:, :, 3], tm[:, :, 0], 4.0)
                        nc.vector.tensor_tensor(v[:, :, 2], tm[:, :, 3], tm[:, :, 1],
                                                AluOpType.subtract)
                        # p = (d3,d4)-(d1,d2); v3 = 2*p1+p2; v4 = p2-2*p1
                        nc.vector.tensor_tensor(pairc(4), dpairc(ct, 3), dpairc(ct, 1),
                                                AluOpType.subtract)
                        nc.vector.tensor_scalar_mul(tm[:, :, 6], tm[:, :, 4], 2.0)
                        nc.vector.tensor_tensor(v[:, :, 3], tm[:, :, 6], tm[:, :, 5],
                                                AluOpType.add)
                        nc.vector.tensor_tensor(v[:, :, 4], tm[:, :, 5], tm[:, :, 6],
                                                AluOpType.subtract)
                        vt.append(v)
                    for ot in range(2):
                        mt = mpp.tile([128, 3 * 6 * 192], F16, name="m", tag="m")
                        mv = mt.rearrange("p (g u c) -> p g u c", u=6, c=192)
                        for gi in range(3):
                            ps = psp.tile([128, 1536], F32, name="ps", tag="ps")
                            for u in range(6):
                                i = 0
                                for kj in range(3):
                                    for ct in range(2):
                                        w_ap = ww_sb[ct][:, ((u * 3 + kj) * 2 + ot) * 128 :
                                                         ((u * 3 + kj) * 2 + ot + 1) * 128]
                                        nc.tensor.matmul(
                                            ps[:, u * 256 : u * 256 + 192 - kj],
                                            w_ap,
                                            vt[ct][:, gi, u, kj:192],
                                            start=(i == 0), stop=(i == 5))
                                        i += 1
                            psv = ps.rearrange("p (u c) -> p u c", c=256)
                            nc.scalar.copy(mv[:, gi], psv[:, :6, :192])
                        # AT combine: y = AT @ M for the half's 3 groups.
                        # yt slots: 0:s12 1:s34 2:d12 3:d34 4:t0 5:q1 6:q2 7:q3
                        ytmp = vtp.tile([128, 3 * 8 * 192], F16, name="yt", tag="yt")
                        yt = ytmp.rearrange("p (g i c) -> p g i c", i=8, c=192)
                        stg = stgp.tile([128, 12 * 192], F16, name="stg", tag="stg")
                        sg4 = stg.rearrange("p (g r c) -> p g r c", r=4, c=192)
                        yrow = lambda i: sg4[:, :, i, :]
                        M = lambda u: mv[:, :, u, :]
                        # (s12,s34) = (M1,M3)+(M2,M4); (d12,d34) = (M1,M3)-(M2,M4)
                        nc.vector.tensor_tensor(yt[:, :, 0:2, :], mv[:, :, 1:4:2, :],
                                                mv[:, :, 2:5:2, :], AluOpType.add)
                        nc.vector.tensor_tensor(yt[:, :, 2:4, :], mv[:, :, 1:4:2, :],
                                                mv[:, :, 2:5:2, :], AluOpType.subtract)
                        nc.vector.tensor_tensor(yt[:, :, 4], M(0), yt[:, :, 0],
                                                AluOpType.add)                      # t0
                        nc.vector.tensor_scalar_mul(yt[:, :, 5], yt[:, :, 3], 2.0)  # q1
                        nc.vector.tensor_scalar_mul(yt[:, :, 6], yt[:, :, 1], 4.0)  # q2
                        nc.vector.tensor_scalar_mul(yt[:, :, 7], yt[:, :, 3], 8.0)  # q3
                        # (y0,y1) = (t0,q1) + (s34,d12)
                        nc.vector.tensor_tensor(sg4[:, :, 0:2, :], yt[:, :, 4:6, :],
                                                yt[:, :, 1:3, :], AluOpType.add)
                        nc.vector.tensor_tensor(yrow(2), yt[:, :, 6], yt[:, :, 0],
                                                AluOpType.add)                      # y2
                        nc.vector.tensor_tensor(yt[:, :, 4], yt[:, :, 7], yt[:, :, 2],
                                                AluOpType.add)                      # y3t
                        nc.vector.tensor_tensor(yrow(3), yt[:, :, 4], M(5),
                                                AluOpType.add)                      # y3
                        sg = stg.rearrange("p (r c) -> p r c", c=192)
                        nc.sync.dma_start(
                            out[ai, b, ot, :, 12 * half : 12 * half + 12, :],
                            sg[:, :, :190])

            def wino_job_f2(ai, b, xo):
                """F(2,3) 1D row-Winograd conv of xo -> 24x190 outputs.
                12 pairs of output rows, processed 2 pairs per chunk (one
                PSUM tile = 8 M_u chains, one evac copy)."""
                xov = [xo[ct][:, :XO_F].rearrange("p (r c) -> p r c", c=192)
                       for ct in range(2)]
                for half in range(2):
                    stg = [stgp.tile([128, 12 * 192], F16, name="stg", tag="stg")
                           for _ in range(2)]
                    sgp = [s.rearrange("p (q r c) -> p q r c", r=2, c=192)
                           for s in stg]
                    for c3 in range(3):
                        p0 = 6 * half + 2 * c3
                        vt = []
                        for ct in range(2):
                            t = vp.tile([128, 2 * 4 * 192], F16, name=f"v{ct}",
                                        tag=f"v{ct}")
                            v = t.rearrange("p (g u c) -> p g u c", u=4, c=192)
                            d = [xov[ct][:, 2 * p0 + a : 2 * p0 + a + 3 : 2, :]
                                 for a in range(4)]
                            nc.vector.tensor_tensor(v[:, :, 0], d[0], d[2], AluOpType.subtract)
                            nc.vector.tensor_tensor(v[:, :, 1], d[1], d[2], AluOpType.add)
                            nc.vector.tensor_tensor(v[:, :, 2], d[2], d[1], AluOpType.subtract)
                            nc.vector.tensor_tensor(v[:, :, 3], d[1], d[3], AluOpType.subtract)
                            vt.append(v)
                        for ot in range(2):
                            mt = mpp.tile([128, 2 * 4 * 192], F16, name="m", tag="m")
                            mv = mt.rearrange("p (g u c) -> p g u c", u=4, c=192)
                            ps = psp.tile([128, 2048], F32, name="ps", tag="ps")
                            for p2 in range(2):
                                for u in range(4):
                                    # slot m=p2*4+u at bank (m//2), half (m%2)
                                    m = p2 * 4 + u
                                    off = (m // 2) * 512 + (m % 2) * 192
                                    i = 0
                                    for kj in range(3):
                                        for ct in range(2):
                                            w_ap = w2_sb[ct][:, ((u * 3 + kj) * 2 + ot) * 128 :
                                                             ((u * 3 + kj) * 2 + ot + 1) * 128]
                                            nc.tensor.matmul(
                                                ps[:, off : off + 192 - kj],
                                                w_ap,
                                                vt[ct][:, p2, u, kj:192],
                                                start=(i == 0), stop=(i == 5))
                                            i += 1
                            psv = ps.rearrange("p (q x) -> p q x", x=512)[:, :, :384]
                            psv = psv.rearrange("p q (m c) -> p q m c", c=192)
                            mvd = mt.rearrange("p (q m c) -> p q m c", q=4, c=192)
                            nc.scalar.copy(mvd, psv)
                            ytmp = vtp.tile([128, 2 * 2 * 192], F16, name="y2", tag="yt")
                            yt = ytmp.rearrange("p (i g c) -> p i g c", g=2, c=192)
                            yrow = lambda i: sgp[ot][:, 2 * c3 : 2 * c3 + 2, i, :]
                            M = lambda u: mv[:, :, u, :]
                            # temps on GpSimd: Vector is saturated in the f2
                            # steady state (V-build + y writes ~= MM window)
                            nc.gpsimd.tensor_tensor(yt[:, 0], M(1), M(2), AluOpType.add)
                            nc.vector.tensor_tensor(yrow(0), M(0), yt[:, 0], AluOpType.add)
                            nc.gpsimd.tensor_tensor(yt[:, 1], M(1), M(2), AluOpType.subtract)
                            nc.vector.tensor_tensor(yrow(1), yt[:, 1], M(3), AluOpType.subtract)
                    for ot in range(2):
                        sg = stg[ot].rearrange("p (r c) -> p r c", c=192)
                        nc.sync.dma_start(
                            out[ai, b, ot, :, 12 * half : 12 * half + 12, :],
                            sg[:, :, :190])

            # angle 0 (no xo build) first for b=0 so the PE starts on the
            # slab DMA alone, and last for b=1 so the tail is the staggered
            # per-rho DMAs of the collapsed job.
            # angle-0 jobs: first for b=0 (PE starts on the slab DMA alone);
            # the b=1 one sits between 90 and 180 so the schedule tail is a
            # Winograd job's short evac+DMA chain, not angle-0's staggered
            # per-rho DMAs.
            conv_job_angle0(0, 0)
            for b in range(2):
                R, C = build_lerp_rc(b)
                xo = build_xo_lerp(45, b, R, C)
                wino_job_f2(1, b, xo)
                xo = build_xo_lerp(135, b, R, C)
                wino_job_f2(3, b, xo)
                xo = build_xo_int(90, b)
                wino_job_f4(2, b, xo)
                if b == 1:
                    conv_job_angle0(0, 1)
                xo = build_xo_int(180, b)
                wino_job_f4(4, b, xo)

    nc.compile()
    return nc


_GRAPH = None


def _graph():
    global _GRAPH
    if _GRAPH is None:
        _GRAPH = build_graph()
    return _GRAPH


def prep_inputs(x, weight):
    x = np.asarray(x, dtype=np.float32)
    weight = np.asarray(weight, dtype=np.float32)
    # pad data rows -2..66, cols -2..67
    xp = np.pad(x, ((0, 0), (0, 0), (2, 3), (2, 4))).astype(np.float16)
    xs_cores = []
    for k in range(NCORES):
        sl = xp[:, :, 8 * k : 8 * k + SLAB_ROWS, :]          # [2,256,13,70]
        sl = sl.reshape(2, 2, 128, SLAB_ROWS * SLAB_COLS)
        xs_cores.append(np.ascontiguousarray(sl))
    w6 = weight.reshape(2, 128, 2, 128, 3, 3)                 # [ot,o,ct,c,ki,kj]

    def wino_w(G):
        nu = G.shape[0]
        wt = np.zeros((nu, 2, 128, 2, 128, 3), np.float32)    # [u,ot,o,ct,c,kj]
        for u in range(nu):
            for ki in range(3):
                wt[u] += G[u, ki] * w6[:, :, :, :, ki, :]
        # -> [ct, c, u, kj, ot, o] -> [2, 128, nu*3*2*128]
        wt = wt.transpose(3, 4, 0, 5, 1, 2).reshape(2, 128, nu * 3 * 2 * 128)
        return np.ascontiguousarray(wt.astype(np.float16))

    wwarr = wino_w(G43)
    w2arr = wino_w(G23)

    combos = []
    for Rr in ROW_COMBOS:
        for Cc in ROW_COMBOS:
            combos.append(w6[..., list(Rr), :][..., list(Cc)].sum(axis=(-1, -2)))
    wcarr = np.stack(combos, axis=0)                          # [16,ot,o,ct,c]
    wcarr = wcarr.transpose(3, 4, 1, 0, 2).reshape(2, 128, 2 * 16 * 128)
    wcarr = np.ascontiguousarray(wcarr.astype(np.float16))
    return xs_cores, wwarr, w2arr, wcarr


def assemble(results):
    full = np.empty((5, 2, 256, NCORES * NR, 190), np.float32)
    for k in range(NCORES):
        o = results[k]["out"].astype(np.float32)              # [5,2,2,128,24,190]
        o = o.reshape(5, 2, 256, NR, 190)
        full[:, :, :, NR * k : NR * (k + 1), :] = o
    full = full[:, :, :, :190, :]
    return tuple(np.ascontiguousarray(full[i]) for i in range(5))


def run(x, weight, trace=False, **trace_kw):
    xs_cores, wwarr, w2arr, wcarr = prep_inputs(x, weight)
    nc = _graph()
    in_maps = [{"xs": xs_cores[k], "ww": wwarr, "w2": w2arr, "wc": wcarr}
               for k in range(NCORES)]
    res = run_bass_kernel_spmd(nc, in_maps, core_ids=list(range(NCORES)),
                               trace=trace, **trace_kw)
    return assemble(res.results), res


def kernel(x, weight):
    return run(x, weight)[0]


# revision 35
# speedup vs baseline: 1.0226x; 1.0226x over previous
"""Trainium2 Bass kernel for nn_AdaptiveAngleConv.

Reference computes, for each of 5 angles, a bilinear "deformable" 3x3
sampling of x (2,256,64,64) into a (2,256,192,192) image, then a 3x3
VALID conv (stride 1) with a shared weight (256,256,3,3), giving 5
outputs of (2,256,190,190).

Key math: the reference's clipped bilinear sampling is exactly an
UNclipped separable 2x2 stencil with constant per-(angle, n)
coefficients on a zero-padded x — every clipped index lands on a
zero-pad row/col, so the clip never changes a nonzero contribution.
Angles 0/90/180 have integer offsets (pure shifted copies); 45/135 need
a 2-pass (rows then cols) lerp.

Sharding: output rows are split across the 8 cores (24 rows each).
Each core receives a pre-sliced 13-row input slab so the SPMD graph is
identical on every core; no collectives.

The conv itself is 1D row-Winograd to cut PE work below the 9
MACs/output-pixel of direct 3x3 conv:
  - angles 90/180: F(4,3)  -> 4.5 MACs/pixel. The sampled image xo is
    built with strided copies on the Scalar engine; the Winograd input
    transform (V = BT @ xo row-windows) runs on Vector with the 2-input
    temps on GpSimd; matmuls accumulate 6 M_u chains per 4-row group in
    PSUM; Scalar evacuates PSUM->SBUF as fp16; Vector applies AT to get
    the 4 output rows.
  - angles 45/135: F(2,3)  -> 6 MACs/pixel (their lerp-based xo build
    needs Vector time, so the cheaper-transform variant keeps Vector
    under the PE window).
  - angle 0: phase-collapsed direct conv (49 taps per 3x3 phase block vs
    81) reading the input slab directly — no sampled image at all.
Outputs are written to DRAM in fp16 (host upcasts); the added ~5e-4
relative error is far inside the 2e-2 gate.

The two angle-0 jobs bracket the schedule (first for batch 0, last for
batch 1) to minimize pipeline head/tail; fp16 keeps the PE at 1
cycle/row with ~8x better rounding than bf16.
"""

import os
import sys

for _p in ("/opt/trn_rl_repo", "/root/.axon_site/_ro/trn_rl_repo"):
    if os.path.isdir(_p) and _p not in sys.path:
        sys.path.insert(0, _p)

import numpy as np

import concourse.bass as bass
import concourse.mybir as mybir
from concourse import bacc, tile
from concourse.alu_op_type import AluOpType
from concourse.bass_utils import run_bass_kernel_spmd

F32 = mybir.dt.float32
F16 = mybir.dt.float16

S2 = 2 ** 0.5
ANGLES = [0, 45, 90, 135, 180]
_OFF = {
    0: ([0.0] * 9, [0.0] * 9),
    1: ([1 - S2, 1 - S2 * 0.5, 1, -S2 * 0.5, 0, S2 * 0.5, -1, S2 * 0.5 - 1, S2 - 1],
        [1, S2 * 0.5, S2 - 1, 1 - S2 * 0.5, 0, S2 * 0.5 - 1, 1 - S2, -S2 * 0.5, -1]),
    2: ([0, 1, 2, -1, 0, 1, -2, -1, 0],
        [2, 1, 0, 1, 0, -1, 0, -1, -2]),
    3: ([1, 1 + S2 * 0.5, 1 + S2, -S2 * 0.5, 0, S2 * 0.5, -1 - S2, -1 - S2 * 0.5, -1],
        [1 + S2, S2 * 0.5, -1, 1 + S2 * 0.5, 0, -1 - S2 * 0.5, 1, -S2 * 0.5, 1 + S2]),
    4: ([2, 2, 2, 0, 0, 0, -2, -2, -2],
        [2, 0, -2, 2, 0, -2, 2, 0, -2]),
}

NCORES = 8
NR = 24            # output rows per core (8*24 = 192, rows 190/191 dropped)
SLAB_ROWS = 13     # input rows a core needs: hi in [8k-2, 8k+10]
SLAB_COLS = 70     # data cols -2..67
XO_ROWS = 26       # NR + 2 halo rows of the sampled image
XO_F = XO_ROWS * 192
XO_ROWS2 = 28      # xo tile rows incl. 2 pad rows (spanned, never read, by
                   # the rows-of-4 rearrange views in wino_job_f4)
XO2F = XO_ROWS2 * 192

# Winograd F(m,3) matrices (Lavin).  BT/AT are encoded directly as the op
# sequences below; G is used on the host for the weight transform.
G43 = np.array([
    [1 / 4, 0, 0],
    [-1 / 6, -1 / 6, -1 / 6],
    [-1 / 6, 1 / 6, -1 / 6],
    [1 / 24, 1 / 12, 1 / 6],
    [1 / 24, -1 / 12, 1 / 6],
    [0, 0, 1]], dtype=np.float32)
G23 = np.array([
    [1, 0, 0],
    [0.5, 0.5, 0.5],
    [0.5, -0.5, 0.5],
    [0, 0, 1]], dtype=np.float32)

WINO_M = {45: 2, 135: 2, 90: 4, 180: 4}   # F(m,3) per angle


def _tables():
    """Per angle: list of (n, r, s, Ax, fx, Ay, fy) in f32 semantics."""
    rng = np.arange(-1, 2)
    pnx, pny = np.meshgrid(rng, rng, indexing="ij")
    pnx = pnx.reshape(-1).astype(np.float32)
    pny = pny.reshape(-1).astype(np.float32)
    out = {}
    for a in ANGLES:
        ox, oy = _OFF[a // 45]
        dx = pnx + np.array(ox, dtype=np.float32)
        dy = pny + np.array(oy, dtype=np.float32)
        rows = []
        for n in range(9):
            Ax = int(np.floor(dx[n]))
            Ay = int(np.floor(dy[n]))
            fx = float(np.float32(dx[n] - Ax))
            fy = float(np.float32(dy[n] - Ay))
            rows.append((n, n // 3, n % 3, Ax, fx, Ay, fy))
        out[a] = rows
    return out


TABLES = _tables()
# distinct fractional row offsets shared by the 45/135 pair
LERP_DS = sorted({(t[3], t[4]) for a in (45, 135) for t in TABLES[a]})

# Angle-0 phase-collapsed conv: output phase rho uses row taps di with the
# listed combo of original kernel rows (g(m)=m//3+m%3-1 collides for m=1,3
# and m=2,4). Combo indices into the host-precomputed sums: 0,1,2 = single
# ki, 3 = ki0+ki2. Same structure for columns. 49 taps/phase-grid vs 81.
ROW_COMBOS = [(0,), (1,), (2,), (0, 2)]
PHROWS = {0: [(-1, 0), (0, 1), (1, 2)],
          1: [(0, 3), (1, 1)],
          2: [(1, 3), (0, 1)]}


def build_graph():
    nc = bacc.Bacc()
    xs = nc.declare_dram_parameter("xs", [2, 2, 128, SLAB_ROWS * SLAB_COLS], F16, False)
    ww = nc.declare_dram_parameter("ww", [2, 128, 6 * 3 * 2 * 128], F16, False)
    w2 = nc.declare_dram_parameter("w2", [2, 128, 4 * 3 * 2 * 128], F16, False)
    wc = nc.declare_dram_parameter("wc", [2, 128, 16 * 2 * 128], F16, False)
    out = nc.declare_dram_parameter("out", [5, 2, 2, 128, NR, 190], F16, True)

    with tile.TileContext(nc) as tc:
        with (
            tc.tile_pool(name="const", bufs=1) as constp,
            tc.tile_pool(name="xop", bufs=2) as xop,
            tc.tile_pool(name="rcp", bufs=1) as rcp,
            tc.tile_pool(name="vp", bufs=2) as vp,
            tc.tile_pool(name="vtmp", bufs=1) as vtp,
            tc.tile_pool(name="mp", bufs=2) as mpp,
            tc.tile_pool(name="stg", bufs=2) as stgp,
            tc.tile_pool(name="ps", bufs=2, space="PSUM") as psp,
        ):
            # HAM warm-up: dependency-free matmuls on an uninitialized tile
            # keep the PE busy during the input-DMA window so the clock gate
            # is already at 8/8 when the first real matmul issues.
            warm = constp.tile([128, 384], F16, name="warm", tag="warm")
            nc.gpsimd.memset(warm[:], 0.0)
            wps = psp.tile([128, 1536], F32, name="wps", tag="ps")
            for _ in range(16):
                nc.tensor.matmul(wps[:, :256], warm[:, :128], warm[:, 128:384],
                                 start=True, stop=True)

            # DMA order matters for the head: the first job (collapsed
            # angle-0, batch 0) needs slab b0 + wc only; w2 is needed one
            # job later, ww three jobs later.
            slab = {}

            def load_slab(b):
                for ct in range(2):
                    s = constp.tile([128, SLAB_ROWS * SLAB_COLS], F16,
                                    name=f"slab{b}{ct}", tag=f"slab{b}{ct}")
                    nc.sync.dma_start(s[:], xs[b, ct])
                    slab[(b, ct)] = s

            load_slab(0)
            # wc is ot-major; load the ot=0 half first so the first job's
            # first matmuls only wait on half the collapsed-weight bytes.
            wc_sb = []
            for ct in range(2):
                wctile = constp.tile([128, 16 * 2 * 128], F16, name=f"wc{ct}",
                                     tag=f"wc{ct}")
                nc.sync.dma_start(wctile[:, :2048], wc[ct][:, :2048])
                wc_sb.append(wctile)
            for ct in range(2):
                nc.sync.dma_start(wc_sb[ct][:, 2048:], wc[ct][:, 2048:])
            w2_sb = []
            for ct in range(2):
                w2t = constp.tile([128, 4 * 3 * 2 * 128], F16, name=f"w2{ct}",
                                  tag=f"w2{ct}")
                nc.sync.dma_start(w2t[:], w2[ct])
                w2_sb.append(w2t)
            load_slab(1)
            ww_sb = []
            for ct in range(2):
                wwt = constp.tile([128, 6 * 3 * 2 * 128], F16, name=f"ww{ct}",
                                  tag=f"ww{ct}")
                nc.sync.dma_start(wwt[:], ww[ct])
                ww_sb.append(wwt)

            def slab3(b, ct):
                return slab[(b, ct)].rearrange("p (r c) -> p r c", c=SLAB_COLS)

            def build_xo_int(a, b):
                """xo tiles for an integer-offset angle via strided copies
                on the Scalar engine (Vector is the scarce resource)."""
                xo = []
                for ct in range(2):
                    t = xop.tile([128, XO2F], F16, name=f"xo{ct}", tag=f"xo{ct}")
                    v = t[:, :XO_F].rearrange("p (r c) -> p r c", c=192)
                    sv = slab3(b, ct)
                    for (n, r, s, Ax, fx, Ay, fy) in TABLES[a]:
                        nrow = 9 if r < 2 else 8
                        src = sv[:, 2 + Ax : 2 + Ax + nrow, 2 + Ay : 66 + Ay]
                        nc.scalar.copy(v[:, r::3, s::3], src)
                    xo.append(t)
                return xo

            # (Ax, fx) row-offsets that some fy!=0 tap reads: only these
            # need a col-diff C tile.
            needs_c = {(t[3], t[4]) for a in (45, 135) for t in TABLES[a]
                       if t[6] != 0.0}

            def build_lerp_rc(b):
                """Shared row-lerp R_d and col-diff C_d tiles for 45+135."""
                R = {}
                C = {}
                for ct in range(2):
                    sv = slab3(b, ct)
                    dr = rcp.tile([128, 12 * SLAB_COLS], F16,
                                  name=f"dr{ct}", tag="dr")
                    drv = dr.rearrange("p (r c) -> p r c", c=SLAB_COLS)
                    nc.vector.tensor_tensor(drv, sv[:, 1:13, :], sv[:, 0:12, :],
                                            AluOpType.subtract)
                    for di, (Ax, fx) in enumerate(LERP_DS):
                        if fx == 0.0:
                            rv = sv[:, 2 + Ax : 11 + Ax, :]
                        else:
                            rt = rcp.tile([128, 9 * SLAB_COLS], F16,
                                          name=f"r{ct}_{di}", tag=f"r{ct}_{di}")
                            rv = rt.rearrange("p (r c) -> p r c", c=SLAB_COLS)
                            nc.vector.scalar_tensor_tensor(
                                rv, drv[:, 2 + Ax : 11 + Ax, :], fx,
                                sv[:, 2 + Ax : 11 + Ax, :],
                                AluOpType.mult, AluOpType.add)
                        R[(ct, Ax, fx)] = rv
                        if (Ax, fx) in needs_c:
                            ctile = rcp.tile([128, 9 * SLAB_COLS], F16,
                                             name=f"c{ct}_{di}", tag=f"c{ct}_{di}")
                            cv = ctile.rearrange("p (r c) -> p r c", c=SLAB_COLS)
                            nc.vector.tensor_tensor(cv[:, :, 0:69], rv[:, :, 1:70],
                                                    rv[:, :, 0:69], AluOpType.subtract)
                            C[(ct, Ax, fx)] = cv
                return R, C

            def build_xo_lerp(a, b, R, C):
                xo = []
                for ct in range(2):
                    t = xop.tile([128, XO2F], F16, name=f"xo{ct}", tag=f"xo{ct}")
                    v = t[:, :XO_F].rearrange("p (r c) -> p r c", c=192)
                    for (n, r, s, Ax, fx, Ay, fy) in TABLES[a]:
                        nrow = 9 if r < 2 else 8
                        rv = R[(ct, Ax, fx)][:, :nrow, 2 + Ay : 66 + Ay]
                        if fy == 0.0:
                            nc.scalar.copy(v[:, r::3, s::3], rv)
                        else:
                            cv = C[(ct, Ax, fx)][:, :nrow, 2 + Ay : 66 + Ay]
                            nc.vector.scalar_tensor_tensor(
                                v[:, r::3, s::3], cv, fy, rv,
                                AluOpType.mult, AluOpType.add)
                    xo.append(t)
                return xo

            def conv_job_angle0(ai, b):
                """Phase-collapsed conv reading the slab directly (no xo).
                Groups by output row phase rho so each rho's rows can DMA
                out as soon as its three column phases are evacuated."""
                for ot in range(2):
                    for rho in range(3):
                        stg = stgp.tile([128, 8 * 192], F16, name="stg", tag="stg")
                        stgv = stg.rearrange("p (r c) -> p r c", c=192)
                        ps = psp.tile([128, 1536], F32, name="ps", tag="ps")
                        for sig in range(3):
                            taps = [(di, dj, ri * 4 + ci)
                                    for (di, ri) in PHROWS[rho]
                                    for (dj, ci) in PHROWS[sig]]
                            nmm = len(taps) * 2
                            i = 0
                            for (di, dj, cb) in taps:
                                for ct in range(2):
                                    sv = slab3(b, ct)
                                    w_ap = wc_sb[ct][:, (ot * 16 + cb) * 128 :
                                                     (ot * 16 + cb + 1) * 128]
                                    nc.tensor.matmul(
                                        ps[:, sig * 512 : (sig + 1) * 512],
                                        w_ap,
                                        sv[:, di + 2 : di + 10, dj + 2 : dj + 66],
                                        start=(i == 0), stop=(i == nmm - 1))
                                    i += 1
                        for sig in range(3):
                            psv = ps[:, sig * 512 : (sig + 1) * 512].rearrange(
                                "p (r c) -> p r c", c=64)
                            nc.scalar.copy(stgv[:, :, sig::3], psv)
                        nc.sync.dma_start(out[ai, b, ot, :, rho::3, :],
                                          stgv[:, :, :190])

            def wino_job_f4(ai, b, xo):
                """F(4,3) 1D row-Winograd conv of xo -> 24x190 outputs.
                6 groups of 4 output rows; per group 6 M_u chains of
                3(kj) x 2(ct) accumulating matmuls; AT combine on Vector.

                scalar_tensor_tensor runs 1x on the DVE, so every scaled
                combine is split into tensor_scalar (4x) + tensor_tensor
                (2x), and adjacent-row operand pairs are fused into one
                [128,3,2,192] op via a rows-of-4 rearrange of xo."""
                xov = [xo[ct][:, :XO_ROWS2 * 192].rearrange("p (r c) -> p r c", c=192)
                       for ct in range(2)]
                for half in range(2):
                    g0 = 3 * half

                    def dpairc(ct, a):
                        """rows (4g+a, 4g+a+1) for the half's 3 groups:
                        [128, 3, 2, 192]."""
                        a0 = 4 * g0 + a
                        v4 = xov[ct][:, a0 : a0 + 12, :].rearrange(
                            "p (g r) c -> p g r c", r=4)
                        return v4[:, :, 0:2, :]

                    def dpair2(ct, a):
                        """rows (4g+a, 4g+a+2): [128, 3, 2, 192]."""
                        a0 = 4 * g0 + a
                        v4 = xov[ct][:, a0 : a0 + 12, :].rearrange(
                            "p (g r) c -> p g r c", r=4)
                        return v4[:, :, 0:3:2, :]

                    vt = []
                    for ct in range(2):
                        t = vp.tile([128, 3 * 6 * 192], F16, name=f"v{ct}",
                                    tag=f"v{ct}")
                        v = t.rearrange("p (g u c) -> p g u c", u=6, c=192)
                        tmp = vtp.tile([128, 3 * 8 * 192], F16, name=f"vt{ct}",
                                       tag=f"vt{ct}")
                        tm = tmp.rearrange("p (g i c) -> p g i c", i=8, c=192)
                        pairc = lambda i: tm[:, :, i : i + 2, :]
                        # ab = -5*(d2,d3); tAtB = ab + (d4,d5); ce = 4*(d0,d1)
                        nc.vector.tensor_scalar_mul(pairc(0), dpairc(ct, 2), -5.0)
                        nc.vector.tensor_tensor(pairc(2), pairc(0), dpairc(ct, 4),
                                                AluOpType.add)
                        nc.vector.tensor_scalar_mul(pairc(4), dpairc(ct, 0), 4.0)
                        # (v0, v5) = ce + tAtB
                        nc.vector.tensor_tensor(v[:, :, 0:6:5, :],
                                                tm[:, :, 4:6, :], tm[:, :, 2:4, :],
                                                AluOpType.add)
                        # s = (d1,d3)+(d2,d4); m = (d1,d3)-(d2,d4)  [GpSimd]
                        nc.gpsimd.tensor_tensor(tm[:, :, 6:8, :], dpair2(ct, 1),
                                                dpair2(ct, 2), AluOpType.add)
                        nc.gpsimd.tensor_tensor(tm[:, :, 0:2, :], dpair2(ct, 1),
                                                dpair2(ct, 2), AluOpType.subtract)
                        # v1 = -4*s1 + s2 ; v2 = 4*m1 - m2
                        nc.vector.tensor_scalar_mul(tm[:, :, 2], tm[:, :, 6], -4.0)
                        nc.vector.tensor_tensor(v[:, :, 1], tm[:, :, 2], tm[:, :, 7],
                                                AluOpType.add)
                        nc.vector.tensor_scalar_mul(tm[:, :, 3], tm[:, :, 0], 4.0)
                        nc.vector.tensor_tensor(v[:, :, 2], tm[:, :, 3], tm[:, :, 1],
                                                AluOpType.subtract)
                        # p = (d3,d4)-(d1,d2); v3 = 2*p1+p2; v4 = p2-2*p1
                        nc.vector.tensor_tensor(pairc(4), dpairc(ct, 3), dpairc(ct, 1),
                                                AluOpType.subtract)
                        nc.vector.tensor_scalar_mul(tm[:, :, 6], tm[:, :, 4], 2.0)
                        nc.vector.tensor_tensor(v[:, :, 3], tm[:, :, 6], tm[:, :, 5],
                                                AluOpType.add)
                        nc.vector.tensor_tensor(v[:, :, 4], tm[:, :, 5], tm[:, :, 6],
                                                AluOpType.subtract)
                        vt.append(v)
                    for ot in range(2):
                        mt = mpp.tile([128, 3 * 6 * 192], F16, name="m", tag="m")
                        mv = mt.rearrange("p (g u c) -> p g u c", u=6, c=192)
                        for gi in range(3):
                            ps = psp.tile([128, 1536], F32, name="ps", tag="ps")
                            for u in range(6):
                                i = 0
                                for kj in range(3):
                                    for ct in range(2):
                                        w_ap = ww_sb[ct][:, ((u * 3 + kj) * 2 + ot) * 128 :
                                                         ((u * 3 + kj) * 2 + ot + 1) * 128]
                                        nc.tensor.matmul(
                                            ps[:, u * 256 : u * 256 + 192 - kj],
                                            w_ap,
                                            vt[ct][:, gi, u, kj:192],
                                            start=(i == 0), stop=(i == 5))
                                        i += 1
                            psv = ps.rearrange("p (u c) -> p u c", c=256)
                            nc.scalar.copy(mv[:, gi], psv[:, :6, :192])
                        # AT combine: y = AT @ M for the half's 3 groups.
                        # yt slots: 0:s12 1:s34 2:d12 3:d34 4:t0 5:q1 6:q2 7:q3
                        ytmp = vtp.tile([128, 3 * 8 * 192], F16, name="yt", tag="yt")
                        yt = ytmp.rearrange("p (g i c) -> p g i c", i=8, c=192)
                        stg = stgp.tile([128, 12 * 192], F16, name="stg", tag="stg")
                        sg4 = stg.rearrange("p (g r c) -> p g r c", r=4, c=192)
                        yrow = lambda i: sg4[:, :, i, :]
                        M = lambda u: mv[:, :, u, :]
                        # (s12,s34) = (M1,M3)+(M2,M4); (d12,d34) = (M1,M3)-(M2,M4)
                        nc.vector.tensor_tensor(yt[:, :, 0:2, :], mv[:, :, 1:4:2, :],
                                                mv[:, :, 2:5:2, :], AluOpType.add)
                        nc.vector.tensor_tensor(yt[:, :, 2:4, :], mv[:, :, 1:4:2, :],
                                                mv[:, :, 2:5:2, :], AluOpType.subtract)
                        nc.vector.tensor_tensor(yt[:, :, 4], M(0), yt[:, :, 0],
                                                AluOpType.add)                      # t0
                        nc.vector.tensor_scalar_mul(yt[:, :, 5], yt[:, :, 3], 2.0)  # q1
                        nc.vector.tensor_scalar_mul(yt[:, :, 6], yt[:, :, 1], 4.0)  # q2
                        nc.vector.tensor_scalar_mul(yt[:, :, 7], yt[:, :, 3], 8.0)  # q3
                        # (y0,y1) = (t0,q1) + (s34,d12)
                        nc.vector.tensor_tensor(sg4[:, :, 0:2, :], yt[:, :, 4:6, :],
                                                yt[:, :, 1:3, :], AluOpType.add)
                        nc.vector.tensor_tensor(yrow(2), yt[:, :, 6], yt[:, :, 0],
                                                AluOpType.add)                      # y2
                        nc.vector.tensor_tensor(yt[:, :, 4], yt[:, :, 7], yt[:, :, 2],
                                                AluOpType.add)                      # y3t
                        nc.vector.tensor_tensor(yrow(3), yt[:, :, 4], M(5),
                                                AluOpType.add)                      # y3
                        sg = stg.rearrange("p (r c) -> p r c", c=192)
                        nc.sync.dma_start(
                            out[ai, b, ot, :, 12 * half : 12 * half + 12, :],
                            sg[:, :, :190])

            def wino_job_f2(ai, b, xo):
                """F(2,3) 1D row-Winograd conv of xo -> 24x190 outputs.
                12 pairs of output rows, processed 2 pairs per chunk (one
                PSUM tile = 8 M_u chains, one evac copy)."""
                xov = [xo[ct][:, :XO_F].rearrange("p (r c) -> p r c", c=192)
                       for ct in range(2)]
                for half in range(2):
                    stg = [stgp.tile([128, 12 * 192], F16, name="stg", tag="stg")
                           for _ in range(2)]
                    sgp = [s.rearrange("p (q r c) -> p q r c", r=2, c=192)
                           for s in stg]
                    for c3 in range(3):
                        p0 = 6 * half + 2 * c3
                        vt = []
                        for ct in range(2):
                            t = vp.tile([128, 2 * 4 * 192], F16, name=f"v{ct}",
                                        tag=f"v{ct}")
                            v = t.rearrange("p (g u c) -> p g u c", u=4, c=192)
                            d = [xov[ct][:, 2 * p0 + a : 2 * p0 + a + 3 : 2, :]
                                 for a in range(4)]
                            nc.vector.tensor_tensor(v[:, :, 0], d[0], d[2], AluOpType.subtract)
                            nc.vector.tensor_tensor(v[:, :, 1], d[1], d[2], AluOpType.add)
                            nc.vector.tensor_tensor(v[:, :, 2], d[2], d[1], AluOpType.subtract)
                            nc.vector.tensor_tensor(v[:, :, 3], d[1], d[3], AluOpType.subtract)
                            vt.append(v)
                        for ot in range(2):
                            mt = mpp.tile([128, 2 * 4 * 192], F16, name="m", tag="m")
                            mv = mt.rearrange("p (g u c) -> p g u c", u=4, c=192)
                            ps = psp.tile([128, 2048], F32, name="ps", tag="ps")
                            for p2 in range(2):
                                for u in range(4):
                                    # slot m=p2*4+u at bank (m//2), half (m%2)
                                    m = p2 * 4 + u
                                    off = (m // 2) * 512 + (m % 2) * 192
                                    i = 0
                                    for kj in range(3):
                                        for ct in range(2):
                                            w_ap = w2_sb[ct][:, ((u * 3 + kj) * 2 + ot) * 128 :
                                                             ((u * 3 + kj) * 2 + ot + 1) * 128]
                                            nc.tensor.matmul(
                                                ps[:, off : off + 192 - kj],
                                                w_ap,
                                                vt[ct][:, p2, u, kj:192],
                                                start=(i == 0), stop=(i == 5))
                                            i += 1
                            psv = ps.rearrange("p (q x) -> p q x", x=512)[:, :, :384]
                            psv = psv.rearrange("p q (m c) -> p q m c", c=192)
                            mvd = mt.rearrange("p (q m c) -> p q m c", q=4, c=192)
                            nc.scalar.copy(mvd, psv)
                            ytmp = vtp.tile([128, 2 * 2 * 192], F16, name="y2", tag="yt")
                            yt = ytmp.rearrange("p (i g c) -> p i g c", g=2, c=192)
                            yrow = lambda i: sgp[ot][:, 2 * c3 : 2 * c3 + 2, i, :]
                            M = lambda u: mv[:, :, u, :]
                            nc.vector.tensor_tensor(yt[:, 0], M(1), M(2), AluOpType.add)
                            nc.vector.tensor_tensor(yrow(0), M(0), yt[:, 0], AluOpType.add)
                            nc.vector.tensor_tensor(yt[:, 1], M(1), M(2), AluOpType.subtract)
                            nc.vector.tensor_tensor(yrow(1), yt[:, 1], M(3), AluOpType.subtract)
                    for ot in range(2):
                        sg = stg[ot].rearrange("p (r c) -> p r c", c=192)
                        nc.sync.dma_start(
                            out[ai, b, ot, :, 12 * half : 12 * half + 12, :],
                            sg[:, :, :190])

            # angle 0 (no xo build) first for b=0 so the PE starts on the
            # slab DMA alone, and last for b=1 so the tail is the staggered
            # per-rho DMAs of the collapsed job.
            # angle-0 jobs: first for b=0 (PE starts on the slab DMA alone);
            # the b=1 one sits between 90 and 180 so the schedule tail is a
            # Winograd job's short evac+DMA chain, not angle-0's staggered
            # per-rho DMAs.
            conv_job_angle0(0, 0)
            for b in range(2):
                R, C = build_lerp_rc(b)
                xo = build_xo_lerp(45, b, R, C)
                wino_job_f2(1, b, xo)
                xo = build_xo_lerp(135, b, R, C)
                wino_job_f2(3, b, xo)
                xo = build_xo_int(90, b)
                wino_job_f4(2, b, xo)
                if b == 1:
                    conv_job_angle0(0, 1)
                xo = build_xo_int(180, b)
                wino_job_f4(4, b, xo)

    nc.compile()
    return nc


_GRAPH = None


def _graph():
    global _GRAPH
    if _GRAPH is None:
        _GRAPH = build_graph()
    return _GRAPH


def prep_inputs(x, weight):
    x = np.asarray(x, dtype=np.float32)
    weight = np.asarray(weight, dtype=np.float32)
    # pad data rows -2..66, cols -2..67
    xp = np.pad(x, ((0, 0), (0, 0), (2, 3), (2, 4))).astype(np.float16)
    xs_cores = []
    for k in range(NCORES):
        sl = xp[:, :, 8 * k : 8 * k + SLAB_ROWS, :]          # [2,256,13,70]
        sl = sl.reshape(2, 2, 128, SLAB_ROWS * SLAB_COLS)
        xs_cores.append(np.ascontiguousarray(sl))
    w6 = weight.reshape(2, 128, 2, 128, 3, 3)                 # [ot,o,ct,c,ki,kj]

    def wino_w(G):
        nu = G.shape[0]
        wt = np.zeros((nu, 2, 128, 2, 128, 3), np.float32)    # [u,ot,o,ct,c,kj]
        for u in range(nu):
            for ki in range(3):
                wt[u] += G[u, ki] * w6[:, :, :, :, ki, :]
        # -> [ct, c, u, kj, ot, o] -> [2, 128, nu*3*2*128]
        wt = wt.transpose(3, 4, 0, 5, 1, 2).reshape(2, 128, nu * 3 * 2 * 128)
        return np.ascontiguousarray(wt.astype(np.float16))

    wwarr = wino_w(G43)
    w2arr = wino_w(G23)

    combos = []
    for Rr in ROW_COMBOS:
        for Cc in ROW_COMBOS:
            combos.append(w6[..., list(Rr), :][..., list(Cc)].sum(axis=(-1, -2)))
    wcarr = np.stack(combos, axis=0)                          # [16,ot,o,ct,c]
    wcarr = wcarr.transpose(3, 4, 1, 0, 2).reshape(2, 128, 2 * 16 * 128)
    wcarr = np.ascontiguousarray(wcarr.astype(np.float16))
    return xs_cores, wwarr, w2arr, wcarr


def assemble(results):
    full = np.empty((5, 2, 256, NCORES * NR, 190), np.float32)
    for k in range(NCORES):
        o = results[k]["out"].astype(np.float32)              # [5,2,2,128,24,190]
        o = o.reshape(5, 2, 256, NR, 190)
        full[:, :, :, NR * k : NR * (k + 1), :] = o
    full = full[:, :, :, :190, :]
    return tuple(np.ascontiguousarray(full[i]) for i in range(5))


def run(x, weight, trace=False, **trace_kw):
    xs_cores, wwarr, w2arr, wcarr = prep_inputs(x, weight)
    nc = _graph()
    in_maps = [{"xs": xs_cores[k], "ww": wwarr, "w2": w2arr, "wc": wcarr}
               for k in range(NCORES)]
    res = run_bass_kernel_spmd(nc, in_maps, core_ids=list(range(NCORES)),
                               trace=trace, **trace_kw)
    return assemble(res.results), res


def kernel(x, weight):
    return run(x, weight)[0]


# revision 36
# speedup vs baseline: 1.0891x; 1.0651x over previous
"""Trainium2 Bass kernel for nn_AdaptiveAngleConv.

Reference computes, for each of 5 angles, a bilinear "deformable" 3x3
sampling of x (2,256,64,64) into a (2,256,192,192) image, then a 3x3
VALID conv (stride 1) with a shared weight (256,256,3,3), giving 5
outputs of (2,256,190,190).

Key math: the reference's clipped bilinear sampling is exactly an
UNclipped separable 2x2 stencil with constant per-(angle, n)
coefficients on a zero-padded x — every clipped index lands on a
zero-pad row/col, so the clip never changes a nonzero contribution.
Angles 0/90/180 have integer offsets (pure shifted copies); 45/135 need
a 2-pass (rows then cols) lerp.

Sharding: output rows are split across the 8 cores (24 rows each).
Each core receives a pre-sliced 13-row input slab so the SPMD graph is
identical on every core; no collectives.

The conv itself is 1D row-Winograd to cut PE work below the 9
MACs/output-pixel of direct 3x3 conv:
  - angles 90/180: F(4,3)  -> 4.5 MACs/pixel. The sampled image xo is
    built with strided copies on the Scalar engine; the Winograd input
    transform (V = BT @ xo row-windows) runs on Vector with the 2-input
    temps on GpSimd; matmuls accumulate 6 M_u chains per 4-row group in
    PSUM; Scalar evacuates PSUM->SBUF as fp16; Vector applies AT to get
    the 4 output rows.
  - angles 45/135: F(2,3)  -> 6 MACs/pixel (their lerp-based xo build
    needs Vector time, so the cheaper-transform variant keeps Vector
    under the PE window).
  - angle 0: phase-collapsed direct conv (49 taps per 3x3 phase block vs
    81) reading the input slab directly — no sampled image at all.
Outputs are written to DRAM in fp16 (host upcasts); the added ~5e-4
relative error is far inside the 2e-2 gate.

The two angle-0 jobs bracket the schedule (first for batch 0, last for
batch 1) to minimize pipeline head/tail; fp16 keeps the PE at 1
cycle/row with ~8x better rounding than bf16.
"""

import os
import sys

for _p in ("/opt/trn_rl_repo", "/root/.axon_site/_ro/trn_rl_repo"):
    if os.path.isdir(_p) and _p not in sys.path:
        sys.path.insert(0, _p)

import numpy as np

import concourse.bass as bass
import concourse.mybir as mybir
from concourse import bacc, tile
from concourse.alu_op_type import AluOpType
from concourse.bass_utils import run_bass_kernel_spmd

F32 = mybir.dt.float32
F16 = mybir.dt.float16

S2 = 2 ** 0.5
ANGLES = [0, 45, 90, 135, 180]
_OFF = {
    0: ([0.0] * 9, [0.0] * 9),
    1: ([1 - S2, 1 - S2 * 0.5, 1, -S2 * 0.5, 0, S2 * 0.5, -1, S2 * 0.5 - 1, S2 - 1],
        [1, S2 * 0.5, S2 - 1, 1 - S2 * 0.5, 0, S2 * 0.5 - 1, 1 - S2, -S2 * 0.5, -1]),
    2: ([0, 1, 2, -1, 0, 1, -2, -1, 0],
        [2, 1, 0, 1, 0, -1, 0, -1, -2]),
    3: ([1, 1 + S2 * 0.5, 1 + S2, -S2 * 0.5, 0, S2 * 0.5, -1 - S2, -1 - S2 * 0.5, -1],
        [1 + S2, S2 * 0.5, -1, 1 + S2 * 0.5, 0, -1 - S2 * 0.5, 1, -S2 * 0.5, 1 + S2]),
    4: ([2, 2, 2, 0, 0, 0, -2, -2, -2],
        [2, 0, -2, 2, 0, -2, 2, 0, -2]),
}

NCORES = 8
NR = 24            # output rows per core (8*24 = 192, rows 190/191 dropped)
SLAB_ROWS = 13     # input rows a core needs: hi in [8k-2, 8k+10]
SLAB_COLS = 70     # data cols -2..67
XO_ROWS = 26       # NR + 2 halo rows of the sampled image
XO_F = XO_ROWS * 192
XO_ROWS2 = 28      # xo tile rows incl. 2 pad rows (spanned, never read, by
                   # the rows-of-4 rearrange views in wino_job_f4)
XO2F = XO_ROWS2 * 192

# Winograd F(m,3) matrices (Lavin).  BT/AT are encoded directly as the op
# sequences below; G is used on the host for the weight transform.
G43 = np.array([
    [1 / 4, 0, 0],
    [-1 / 6, -1 / 6, -1 / 6],
    [-1 / 6, 1 / 6, -1 / 6],
    [1 / 24, 1 / 12, 1 / 6],
    [1 / 24, -1 / 12, 1 / 6],
    [0, 0, 1]], dtype=np.float32)
G23 = np.array([
    [1, 0, 0],
    [0.5, 0.5, 0.5],
    [0.5, -0.5, 0.5],
    [0, 0, 1]], dtype=np.float32)

WINO_M = {45: 2, 135: 2, 90: 4, 180: 4}   # F(m,3) per angle


def _tables():
    """Per angle: list of (n, r, s, Ax, fx, Ay, fy) in f32 semantics."""
    rng = np.arange(-1, 2)
    pnx, pny = np.meshgrid(rng, rng, indexing="ij")
    pnx = pnx.reshape(-1).astype(np.float32)
    pny = pny.reshape(-1).astype(np.float32)
    out = {}
    for a in ANGLES:
        ox, oy = _OFF[a // 45]
        dx = pnx + np.array(ox, dtype=np.float32)
        dy = pny + np.array(oy, dtype=np.float32)
        rows = []
        for n in range(9):
            Ax = int(np.floor(dx[n]))
            Ay = int(np.floor(dy[n]))
            fx = float(np.float32(dx[n] - Ax))
            fy = float(np.float32(dy[n] - Ay))
            rows.append((n, n // 3, n % 3, Ax, fx, Ay, fy))
        out[a] = rows
    return out


TABLES = _tables()
# distinct fractional row offsets shared by the 45/135 pair
LERP_DS = sorted({(t[3], t[4]) for a in (45, 135) for t in TABLES[a]})

# Angle-0 phase-collapsed conv: output phase rho uses row taps di with the
# listed combo of original kernel rows (g(m)=m//3+m%3-1 collides for m=1,3
# and m=2,4). Combo indices into the host-precomputed sums: 0,1,2 = single
# ki, 3 = ki0+ki2. Same structure for columns. 49 taps/phase-grid vs 81.
ROW_COMBOS = [(0,), (1,), (2,), (0, 2)]
PHROWS = {0: [(-1, 0), (0, 1), (1, 2)],
          1: [(0, 3), (1, 1)],
          2: [(1, 3), (0, 1)]}


def build_graph():
    nc = bacc.Bacc()
    xs = nc.declare_dram_parameter("xs", [2, 2, 128, SLAB_ROWS * SLAB_COLS], F16, False)
    ww = nc.declare_dram_parameter("ww", [2, 128, 6 * 3 * 2 * 128], F16, False)
    w2 = nc.declare_dram_parameter("w2", [2, 128, 4 * 3 * 2 * 128], F16, False)
    wc = nc.declare_dram_parameter("wc", [2, 128, 16 * 2 * 128], F16, False)
    out = nc.declare_dram_parameter("out", [5, 2, 2, 128, NR, 190], F16, True)

    with tile.TileContext(nc) as tc:
        with (
            tc.tile_pool(name="const", bufs=1) as constp,
            tc.tile_pool(name="xop", bufs=2) as xop,
            tc.tile_pool(name="rcp", bufs=1) as rcp,
            tc.tile_pool(name="vp", bufs=2) as vp,
            tc.tile_pool(name="vtmp", bufs=1) as vtp,
            tc.tile_pool(name="mp", bufs=2) as mpp,
            tc.tile_pool(name="stg", bufs=2) as stgp,
            tc.tile_pool(name="ps", bufs=2, space="PSUM") as psp,
        ):
            # HAM warm-up: dependency-free matmuls on an uninitialized tile
            # keep the PE busy during the input-DMA window so the clock gate
            # is already at 8/8 when the first real matmul issues.
            warm = constp.tile([128, 384], F16, name="warm", tag="warm")
            nc.gpsimd.memset(warm[:], 0.0)
            wps = psp.tile([128, 1536], F32, name="wps", tag="ps")
            for _ in range(16):
                nc.tensor.matmul(wps[:, :256], warm[:, :128], warm[:, 128:384],
                                 start=True, stop=True)

            # DMA order matters for the head: the first job (collapsed
            # angle-0, batch 0) needs slab b0 + wc only; w2 is needed one
            # job later, ww three jobs later.
            slab = {}

            def load_slab(b):
                for ct in range(2):
                    s = constp.tile([128, SLAB_ROWS * SLAB_COLS], F16,
                                    name=f"slab{b}{ct}", tag=f"slab{b}{ct}")
                    nc.sync.dma_start(s[:], xs[b, ct])
                    slab[(b, ct)] = s

            load_slab(0)
            # wc is ot-major; load the ot=0 half first so the first job's
            # first matmuls only wait on half the collapsed-weight bytes.
            wc_sb = []
            for ct in range(2):
                wctile = constp.tile([128, 16 * 2 * 128], F16, name=f"wc{ct}",
                                     tag=f"wc{ct}")
                nc.sync.dma_start(wctile[:, :2048], wc[ct][:, :2048])
                wc_sb.append(wctile)
            for ct in range(2):
                nc.sync.dma_start(wc_sb[ct][:, 2048:], wc[ct][:, 2048:])
            w2_sb = []
            for ct in range(2):
                w2t = constp.tile([128, 4 * 3 * 2 * 128], F16, name=f"w2{ct}",
                                  tag=f"w2{ct}")
                nc.sync.dma_start(w2t[:], w2[ct])
                w2_sb.append(w2t)
            load_slab(1)
            ww_sb = []
            for ct in range(2):
                wwt = constp.tile([128, 6 * 3 * 2 * 128], F16, name=f"ww{ct}",
                                  tag=f"ww{ct}")
                nc.sync.dma_start(wwt[:], ww[ct])
                ww_sb.append(wwt)

            def slab3(b, ct):
                return slab[(b, ct)].rearrange("p (r c) -> p r c", c=SLAB_COLS)

            def build_xo_int(a, b):
                """xo tiles for an integer-offset angle via strided copies
                on the Scalar engine (Vector is the scarce resource)."""
                xo = []
                for ct in range(2):
                    t = xop.tile([128, XO2F], F16, name=f"xo{ct}", tag=f"xo{ct}")
                    v = t[:, :XO_F].rearrange("p (r c) -> p r c", c=192)
                    sv = slab3(b, ct)
                    for (n, r, s, Ax, fx, Ay, fy) in TABLES[a]:
                        nrow = 9 if r < 2 else 8
                        src = sv[:, 2 + Ax : 2 + Ax + nrow, 2 + Ay : 66 + Ay]
                        nc.scalar.copy(v[:, r::3, s::3], src)
                    xo.append(t)
                return xo

            # (Ax, fx) row-offsets that some fy!=0 tap reads: only these
            # need a col-diff C tile.
            needs_c = {(t[3], t[4]) for a in (45, 135) for t in TABLES[a]
                       if t[6] != 0.0}

            def build_lerp_rc(b):
                """Shared row-lerp R_d and col-diff C_d tiles for 45+135."""
                R = {}
                C = {}
                for ct in range(2):
                    sv = slab3(b, ct)
                    dr = rcp.tile([128, 12 * SLAB_COLS], F16,
                                  name=f"dr{ct}", tag="dr")
                    drv = dr.rearrange("p (r c) -> p r c", c=SLAB_COLS)
                    nc.vector.tensor_tensor(drv, sv[:, 1:13, :], sv[:, 0:12, :],
                                            AluOpType.subtract)
                    for di, (Ax, fx) in enumerate(LERP_DS):
                        if fx == 0.0:
                            rv = sv[:, 2 + Ax : 11 + Ax, :]
                        else:
                            rt = rcp.tile([128, 9 * SLAB_COLS], F16,
                                          name=f"r{ct}_{di}", tag=f"r{ct}_{di}")
                            rv = rt.rearrange("p (r c) -> p r c", c=SLAB_COLS)
                            nc.vector.scalar_tensor_tensor(
                                rv, drv[:, 2 + Ax : 11 + Ax, :], fx,
                                sv[:, 2 + Ax : 11 + Ax, :],
                                AluOpType.mult, AluOpType.add)
                        R[(ct, Ax, fx)] = rv
                        if (Ax, fx) in needs_c:
                            ctile = rcp.tile([128, 9 * SLAB_COLS], F16,
                                             name=f"c{ct}_{di}", tag=f"c{ct}_{di}")
                            cv = ctile.rearrange("p (r c) -> p r c", c=SLAB_COLS)
                            nc.vector.tensor_tensor(cv[:, :, 0:69], rv[:, :, 1:70],
                                                    rv[:, :, 0:69], AluOpType.subtract)
                            C[(ct, Ax, fx)] = cv
                return R, C

            def build_xo_lerp(a, b, R, C):
                xo = []
                for ct in range(2):
                    t = xop.tile([128, XO2F], F16, name=f"xo{ct}", tag=f"xo{ct}")
                    v = t[:, :XO_F].rearrange("p (r c) -> p r c", c=192)
                    for (n, r, s, Ax, fx, Ay, fy) in TABLES[a]:
                        nrow = 9 if r < 2 else 8
                        rv = R[(ct, Ax, fx)][:, :nrow, 2 + Ay : 66 + Ay]
                        if fy == 0.0:
                            nc.scalar.copy(v[:, r::3, s::3], rv)
                        else:
                            cv = C[(ct, Ax, fx)][:, :nrow, 2 + Ay : 66 + Ay]
                            nc.vector.scalar_tensor_tensor(
                                v[:, r::3, s::3], cv, fy, rv,
                                AluOpType.mult, AluOpType.add)
                    xo.append(t)
                return xo

            def conv_job_angle0(ai, b):
                """Phase-collapsed conv reading the slab directly (no xo).
                Groups by output row phase rho so each rho's rows can DMA
                out as soon as its three column phases are evacuated."""
                for ot in range(2):
                    for rho in range(3):
                        stg = stgp.tile([128, 8 * 192], F16, name="stg", tag="stg")
                        stgv = stg.rearrange("p (r c) -> p r c", c=192)
                        ps = psp.tile([128, 1536], F32, name="ps", tag="ps")
                        for sig in range(3):
                            taps = [(di, dj, ri * 4 + ci)
                                    for (di, ri) in PHROWS[rho]
                                    for (dj, ci) in PHROWS[sig]]
                            nmm = len(taps) * 2
                            i = 0
                            for (di, dj, cb) in taps:
                                for ct in range(2):
                                    sv = slab3(b, ct)
                                    w_ap = wc_sb[ct][:, (ot * 16 + cb) * 128 :
                                                     (ot * 16 + cb + 1) * 128]
                                    nc.tensor.matmul(
                                        ps[:, sig * 512 : (sig + 1) * 512],
                                        w_ap,
                                        sv[:, di + 2 : di + 10, dj + 2 : dj + 66],
                                        start=(i == 0), stop=(i == nmm - 1))
                                    i += 1
                        for sig in range(3):
                            psv = ps[:, sig * 512 : (sig + 1) * 512].rearrange(
                                "p (r c) -> p r c", c=64)
                            nc.scalar.copy(stgv[:, :, sig::3], psv)
                        nc.sync.dma_start(out[ai, b, ot, :, rho::3, :],
                                          stgv[:, :, :190])

            def wino_job_f4(ai, b, xo):
                """F(4,3) 1D row-Winograd conv of xo -> 24x190 outputs.
                6 groups of 4 output rows; per group 6 M_u chains of
                3(kj) x 2(ct) accumulating matmuls; AT combine on Vector.

                scalar_tensor_tensor runs 1x on the DVE, so every scaled
                combine is split into tensor_scalar (4x) + tensor_tensor
                (2x), and adjacent-row operand pairs are fused into one
                [128,3,2,192] op via a rows-of-4 rearrange of xo."""
                xov = [xo[ct][:, :XO_ROWS2 * 192].rearrange("p (r c) -> p r c", c=192)
                       for ct in range(2)]
                for half in range(2):
                    g0 = 3 * half

                    def dpairc(ct, a):
                        """rows (4g+a, 4g+a+1) for the half's 3 groups:
                        [128, 3, 2, 192]."""
                        a0 = 4 * g0 + a
                        v4 = xov[ct][:, a0 : a0 + 12, :].rearrange(
                            "p (g r) c -> p g r c", r=4)
                        return v4[:, :, 0:2, :]

                    def dpair2(ct, a):
                        """rows (4g+a, 4g+a+2): [128, 3, 2, 192]."""
                        a0 = 4 * g0 + a
                        v4 = xov[ct][:, a0 : a0 + 12, :].rearrange(
                            "p (g r) c -> p g r c", r=4)
                        return v4[:, :, 0:3:2, :]

                    vt = []
                    for ct in range(2):
                        t = vp.tile([128, 3 * 6 * 192], F16, name=f"v{ct}",
                                    tag=f"v{ct}")
                        v = t.rearrange("p (g u c) -> p g u c", u=6, c=192)
                        tmp = vtp.tile([128, 3 * 8 * 192], F16, name=f"vt{ct}",
                                       tag=f"vt{ct}")
                        tm = tmp.rearrange("p (g i c) -> p g i c", i=8, c=192)
                        pairc = lambda i: tm[:, :, i : i + 2, :]
                        # ab = -5*(d2,d3); tAtB = ab + (d4,d5); ce = 4*(d0,d1)
                        nc.vector.tensor_scalar_mul(pairc(0), dpairc(ct, 2), -5.0)
                        nc.vector.tensor_tensor(pairc(2), pairc(0), dpairc(ct, 4),
                                                AluOpType.add)
                        nc.vector.tensor_scalar_mul(pairc(4), dpairc(ct, 0), 4.0)
                        # (v0, v5) = ce + tAtB
                        nc.vector.tensor_tensor(v[:, :, 0:6:5, :],
                                                tm[:, :, 4:6, :], tm[:, :, 2:4, :],
                                                AluOpType.add)
                        # s = (d1,d3)+(d2,d4); m = (d1,d3)-(d2,d4)  [GpSimd]
                        nc.gpsimd.tensor_tensor(tm[:, :, 6:8, :], dpair2(ct, 1),
                                                dpair2(ct, 2), AluOpType.add)
                        nc.gpsimd.tensor_tensor(tm[:, :, 0:2, :], dpair2(ct, 1),
                                                dpair2(ct, 2), AluOpType.subtract)
                        # v1 = -4*s1 + s2 ; v2 = 4*m1 - m2
                        nc.vector.tensor_scalar_mul(tm[:, :, 2], tm[:, :, 6], -4.0)
                        nc.vector.tensor_tensor(v[:, :, 1], tm[:, :, 2], tm[:, :, 7],
                                                AluOpType.add)
                        nc.vector.tensor_scalar_mul(tm[:, :, 3], tm[:, :, 0], 4.0)
                        nc.vector.tensor_tensor(v[:, :, 2], tm[:, :, 3], tm[:, :, 1],
                                                AluOpType.subtract)
                        # p = (d3,d4)-(d1,d2); v3 = 2*p1+p2; v4 = p2-2*p1
                        nc.vector.tensor_tensor(pairc(4), dpairc(ct, 3), dpairc(ct, 1),
                                                AluOpType.subtract)
                        nc.vector.tensor_scalar_mul(tm[:, :, 6], tm[:, :, 4], 2.0)
                        nc.vector.tensor_tensor(v[:, :, 3], tm[:, :, 6], tm[:, :, 5],
                                                AluOpType.add)
                        nc.vector.tensor_tensor(v[:, :, 4], tm[:, :, 5], tm[:, :, 6],
                                                AluOpType.subtract)
                        vt.append(v)
                    for ot in range(2):
                        mt = mpp.tile([128, 3 * 6 * 192], F16, name="m", tag="m")
                        mv = mt.rearrange("p (g u c) -> p g u c", u=6, c=192)
                        for gi in range(3):
                            ps = psp.tile([128, 1536], F32, name="ps", tag="ps")
                            for u in range(6):
                                i = 0
                                for kj in range(3):
                                    for ct in range(2):
                                        w_ap = ww_sb[ct][:, ((u * 3 + kj) * 2 + ot) * 128 :
                                                         ((u * 3 + kj) * 2 + ot + 1) * 128]
                                        nc.tensor.matmul(
                                            ps[:, u * 256 : u * 256 + 192 - kj],
                                            w_ap,
                                            vt[ct][:, gi, u, kj:192],
                                            start=(i == 0), stop=(i == 5))
                                        i += 1
                            psv = ps.rearrange("p (u c) -> p u c", c=256)
                            nc.scalar.copy(mv[:, gi], psv[:, :6, :192])
                        # AT combine: y = AT @ M for the half's 3 groups.
                        # yt slots: 0:s12 1:s34 2:d12 3:d34 4:t0 5:q1 6:q2 7:q3
                        ytmp = vtp.tile([128, 3 * 8 * 192], F16, name="yt", tag="yt")
                        yt = ytmp.rearrange("p (g i c) -> p g i c", i=8, c=192)
                        stg = stgp.tile([128, 12 * 192], F16, name="stg", tag="stg")
                        sg4 = stg.rearrange("p (g r c) -> p g r c", r=4, c=192)
                        yrow = lambda i: sg4[:, :, i, :]
                        M = lambda u: mv[:, :, u, :]
                        # (s12,s34) = (M1,M3)+(M2,M4); (d12,d34) = (M1,M3)-(M2,M4)
                        nc.vector.tensor_tensor(yt[:, :, 0:2, :], mv[:, :, 1:4:2, :],
                                                mv[:, :, 2:5:2, :], AluOpType.add)
                        nc.vector.tensor_tensor(yt[:, :, 2:4, :], mv[:, :, 1:4:2, :],
                                                mv[:, :, 2:5:2, :], AluOpType.subtract)
                        nc.vector.tensor_tensor(yt[:, :, 4], M(0), yt[:, :, 0],
                                                AluOpType.add)                      # t0
                        nc.vector.tensor_scalar_mul(yt[:, :, 5], yt[:, :, 3], 2.0)  # q1
                        nc.vector.tensor_scalar_mul(yt[:, :, 6], yt[:, :, 1], 4.0)  # q2
                        nc.vector.tensor_scalar_mul(yt[:, :, 7], yt[:, :, 3], 8.0)  # q3
                        # (y0,y1) = (t0,q1) + (s34,d12)
                        nc.vector.tensor_tensor(sg4[:, :, 0:2, :], yt[:, :, 4:6, :],
                                                yt[:, :, 1:3, :], AluOpType.add)
                        nc.vector.tensor_tensor(yrow(2), yt[:, :, 6], yt[:, :, 0],
                                                AluOpType.add)                      # y2
                        nc.vector.tensor_tensor(yt[:, :, 4], yt[:, :, 7], yt[:, :, 2],
                                                AluOpType.add)                      # y3t
                        nc.vector.tensor_tensor(yrow(3), yt[:, :, 4], M(5),
                                                AluOpType.add)                      # y3
                        sg = stg.rearrange("p (r c) -> p r c", c=192)
                        nc.sync.dma_start(
                            out[ai, b, ot, :, 12 * half : 12 * half + 12, :],
                            sg[:, :, :190])

            def wino_job_f2(ai, b, xo):
                """F(2,3) 1D row-Winograd conv of xo -> 24x190 outputs.
                12 pairs of output rows, processed 2 pairs per chunk (one
                PSUM tile = 8 M_u chains, one evac copy)."""
                xov = [xo[ct][:, :XO_F].rearrange("p (r c) -> p r c", c=192)
                       for ct in range(2)]
                for half in range(2):
                    stg = [stgp.tile([128, 12 * 192], F16, name="stg", tag="stg")
                           for _ in range(2)]
                    sgp = [s.rearrange("p (q r c) -> p q r c", r=2, c=192)
                           for s in stg]
                    for c3 in range(3):
                        p0 = 6 * half + 2 * c3
                        vt = []
                        for ct in range(2):
                            t = vp.tile([128, 2 * 4 * 192], F16, name=f"v{ct}",
                                        tag=f"v{ct}")
                            v = t.rearrange("p (g u c) -> p g u c", u=4, c=192)
                            d = [xov[ct][:, 2 * p0 + a : 2 * p0 + a + 3 : 2, :]
                                 for a in range(4)]
                            nc.vector.tensor_tensor(v[:, :, 0], d[0], d[2], AluOpType.subtract)
                            nc.vector.tensor_tensor(v[:, :, 1], d[1], d[2], AluOpType.add)
                            nc.vector.tensor_tensor(v[:, :, 2], d[2], d[1], AluOpType.subtract)
                            nc.vector.tensor_tensor(v[:, :, 3], d[1], d[3], AluOpType.subtract)
                            vt.append(v)
                        for ot in range(2):
                            mt = mpp.tile([128, 2 * 4 * 192], F16, name="m", tag="m")
                            mv = mt.rearrange("p (g u c) -> p g u c", u=4, c=192)
                            ps = psp.tile([128, 2048], F32, name="ps", tag="ps")
                            for p2 in range(2):
                                for u in range(4):
                                    # slot m=p2*4+u at bank (m//2), half (m%2)
                                    m = p2 * 4 + u
                                    off = (m // 2) * 512 + (m % 2) * 192
                                    i = 0
                                    for kj in range(3):
                                        for ct in range(2):
                                            w_ap = w2_sb[ct][:, ((u * 3 + kj) * 2 + ot) * 128 :
                                                             ((u * 3 + kj) * 2 + ot + 1) * 128]
                                            nc.tensor.matmul(
                                                ps[:, off : off + 192 - kj],
                                                w_ap,
                                                vt[ct][:, p2, u, kj:192],
                                                start=(i == 0), stop=(i == 5))
                                            i += 1
                            psv = ps.rearrange("p (q x) -> p q x", x=512)[:, :, :384]
                            psv = psv.rearrange("p q (m c) -> p q m c", c=192)
                            mvd = mt.rearrange("p (q m c) -> p q m c", q=4, c=192)
                            nc.scalar.copy(mvd, psv)
                            ytmp = vtp.tile([128, 2 * 2 * 192], F16, name="y2", tag="yt")
                            yt = ytmp.rearrange("p (i g c) -> p i g c", g=2, c=192)
                            yrow = lambda i: sgp[ot][:, 2 * c3 : 2 * c3 + 2, i, :]
                            M = lambda u: mv[:, :, u, :]
                            nc.vector.tensor_tensor(yt[:, 0], M(1), M(2), AluOpType.add)
                            nc.vector.tensor_tensor(yrow(0), M(0), yt[:, 0], AluOpType.add)
                            nc.vector.tensor_tensor(yt[:, 1], M(1), M(2), AluOpType.subtract)
                            nc.vector.tensor_tensor(yrow(1), yt[:, 1], M(3), AluOpType.subtract)
                    for ot in range(2):
                        sg = stg[ot].rearrange("p (r c) -> p r c", c=192)
                        nc.sync.dma_start(
                            out[ai, b, ot, :, 12 * half : 12 * half + 12, :],
                            sg[:, :, :190])

            # angle 0 (no xo build) first for b=0 so the PE starts on the
            # slab DMA alone, and last for b=1 so the tail is the staggered
            # per-rho DMAs of the collapsed job.
            conv_job_angle0(0, 0)
            for b in range(2):
                R, C = build_lerp_rc(b)
                xo = build_xo_lerp(45, b, R, C)
                wino_job_f2(1, b, xo)
                xo = build_xo_lerp(135, b, R, C)
                wino_job_f2(3, b, xo)
                xo = build_xo_int(90, b)
                wino_job_f2(2, b, xo)
                xo = build_xo_int(180, b)
                wino_job_f2(4, b, xo)
            conv_job_angle0(0, 1)

    nc.compile()
    return nc


_GRAPH = None


def _graph():
    global _GRAPH
    if _GRAPH is None:
        _GRAPH = build_graph()
    return _GRAPH


def prep_inputs(x, weight):
    x = np.asarray(x, dtype=np.float32)
    weight = np.asarray(weight, dtype=np.float32)
    # pad data rows -2..66, cols -2..67
    xp = np.pad(x, ((0, 0), (0, 0), (2, 3), (2, 4))).astype(np.float16)
    xs_cores = []
    for k in range(NCORES):
        sl = xp[:, :, 8 * k : 8 * k + SLAB_ROWS, :]          # [2,256,13,70]
        sl = sl.reshape(2, 2, 128, SLAB_ROWS * SLAB_COLS)
        xs_cores.append(np.ascontiguousarray(sl))
    w6 = weight.reshape(2, 128, 2, 128, 3, 3)                 # [ot,o,ct,c,ki,kj]

    def wino_w(G):
        nu = G.shape[0]
        wt = np.zeros((nu, 2, 128, 2, 128, 3), np.float32)    # [u,ot,o,ct,c,kj]
        for u in range(nu):
            for ki in range(3):
                wt[u] += G[u, ki] * w6[:, :, :, :, ki, :]
        # -> [ct, c, u, kj, ot, o] -> [2, 128, nu*3*2*128]
        wt = wt.transpose(3, 4, 0, 5, 1, 2).reshape(2, 128, nu * 3 * 2 * 128)
        return np.ascontiguousarray(wt.astype(np.float16))

    wwarr = wino_w(G43)
    w2arr = wino_w(G23)

    combos = []
    for Rr in ROW_COMBOS:
        for Cc in ROW_COMBOS:
            combos.append(w6[..., list(Rr), :][..., list(Cc)].sum(axis=(-1, -2)))
    wcarr = np.stack(combos, axis=0)                          # [16,ot,o,ct,c]
    wcarr = wcarr.transpose(3, 4, 1, 0, 2).reshape(2, 128, 2 * 16 * 128)
    wcarr = np.ascontiguousarray(wcarr.astype(np.float16))
    return xs_cores, wwarr, w2arr, wcarr


def assemble(results):
    full = np.empty((5, 2, 256, NCORES * NR, 190), np.float32)
    for k in range(NCORES):
        o = results[k]["out"].astype(np.float32)              # [5,2,2,128,24,190]
        o = o.reshape(5, 2, 256, NR, 190)
        full[:, :, :, NR * k : NR * (k + 1), :] = o
    full = full[:, :, :, :190, :]
    return tuple(np.ascontiguousarray(full[i]) for i in range(5))


def run(x, weight, trace=False, **trace_kw):
    xs_cores, wwarr, w2arr, wcarr = prep_inputs(x, weight)
    nc = _graph()
    in_maps = [{"xs": xs_cores[k], "ww": wwarr, "w2": w2arr, "wc": wcarr}
               for k in range(NCORES)]
    res = run_bass_kernel_spmd(nc, in_maps, core_ids=list(range(NCORES)),
                               trace=trace, **trace_kw)
    return assemble(res.results), res


def kernel(x, weight):
    return run(x, weight)[0]


# revision 41
# speedup vs baseline: 1.0891x; 1.0001x over previous
"""Trainium2 Bass kernel for nn_AdaptiveAngleConv.

Reference computes, for each of 5 angles, a bilinear "deformable" 3x3
sampling of x (2,256,64,64) into a (2,256,192,192) image, then a 3x3
VALID conv (stride 1) with a shared weight (256,256,3,3), giving 5
outputs of (2,256,190,190).

Key math: the reference's clipped bilinear sampling is exactly an
UNclipped separable 2x2 stencil with constant per-(angle, n)
coefficients on a zero-padded x — every clipped index lands on a
zero-pad row/col, so the clip never changes a nonzero contribution.
Angles 0/90/180 have integer offsets (pure shifted copies); 45/135 need
a 2-pass (rows then cols) lerp.

Sharding: output rows are split across the 8 cores (24 rows each).
Each core receives a pre-sliced 13-row input slab so the SPMD graph is
identical on every core; no collectives.

The conv itself is 1D row-Winograd to cut PE work below the 9
MACs/output-pixel of direct 3x3 conv:
  - angles 90/180: F(4,3)  -> 4.5 MACs/pixel. The sampled image xo is
    built with strided copies on the Scalar engine; the Winograd input
    transform (V = BT @ xo row-windows) runs on Vector with the 2-input
    temps on GpSimd; matmuls accumulate 6 M_u chains per 4-row group in
    PSUM; Scalar evacuates PSUM->SBUF as fp16; Vector applies AT to get
    the 4 output rows.
  - angles 45/135: F(2,3)  -> 6 MACs/pixel (their lerp-based xo build
    needs Vector time, so the cheaper-transform variant keeps Vector
    under the PE window).
  - angle 0: phase-collapsed direct conv (49 taps per 3x3 phase block vs
    81) reading the input slab directly — no sampled image at all.
Outputs are written to DRAM in fp16 (host upcasts); the added ~5e-4
relative error is far inside the 2e-2 gate.

The two angle-0 jobs bracket the schedule (first for batch 0, last for
batch 1) to minimize pipeline head/tail; fp16 keeps the PE at 1
cycle/row with ~8x better rounding than bf16.
"""

import os
import sys

for _p in ("/opt/trn_rl_repo", "/root/.axon_site/_ro/trn_rl_repo"):
    if os.path.isdir(_p) and _p not in sys.path:
        sys.path.insert(0, _p)

import numpy as np

import concourse.bass as bass
import concourse.mybir as mybir
from concourse import bacc, tile
from concourse.alu_op_type import AluOpType
from concourse.bass_utils import run_bass_kernel_spmd

F32 = mybir.dt.float32
F16 = mybir.dt.float16

S2 = 2 ** 0.5
ANGLES = [0, 45, 90, 135, 180]
_OFF = {
    0: ([0.0] * 9, [0.0] * 9),
    1: ([1 - S2, 1 - S2 * 0.5, 1, -S2 * 0.5, 0, S2 * 0.5, -1, S2 * 0.5 - 1, S2 - 1],
        [1, S2 * 0.5, S2 - 1, 1 - S2 * 0.5, 0, S2 * 0.5 - 1, 1 - S2, -S2 * 0.5, -1]),
    2: ([0, 1, 2, -1, 0, 1, -2, -1, 0],
        [2, 1, 0, 1, 0, -1, 0, -1, -2]),
    3: ([1, 1 + S2 * 0.5, 1 + S2, -S2 * 0.5, 0, S2 * 0.5, -1 - S2, -1 - S2 * 0.5, -1],
        [1 + S2, S2 * 0.5, -1, 1 + S2 * 0.5, 0, -1 - S2 * 0.5, 1, -S2 * 0.5, 1 + S2]),
    4: ([2, 2, 2, 0, 0, 0, -2, -2, -2],
        [2, 0, -2, 2, 0, -2, 2, 0, -2]),
}

NCORES = 8
NR = 24            # output rows per core (8*24 = 192, rows 190/191 dropped)
SLAB_ROWS = 13     # input rows a core needs: hi in [8k-2, 8k+10]
SLAB_COLS = 70     # data cols -2..67
XO_ROWS = 26       # NR + 2 halo rows of the sampled image
XO_F = XO_ROWS * 192
XO_ROWS2 = 28      # xo tile rows incl. 2 pad rows (spanned, never read, by
                   # the rows-of-4 rearrange views in wino_job_f4)
XO2F = XO_ROWS2 * 192

# Winograd F(m,3) matrices (Lavin).  BT/AT are encoded directly as the op
# sequences below; G is used on the host for the weight transform.
G43 = np.array([
    [1 / 4, 0, 0],
    [-1 / 6, -1 / 6, -1 / 6],
    [-1 / 6, 1 / 6, -1 / 6],
    [1 / 24, 1 / 12, 1 / 6],
    [1 / 24, -1 / 12, 1 / 6],
    [0, 0, 1]], dtype=np.float32)
G23 = np.array([
    [1, 0, 0],
    [0.5, 0.5, 0.5],
    [-0.5, 0.5, -0.5],    # u2 row negated: the kernel builds d1-d2, not d2-d1
    [0, 0, 1]], dtype=np.float32)

WINO_M = {45: 2, 135: 2, 90: 4, 180: 4}   # F(m,3) per angle


def _tables():
    """Per angle: list of (n, r, s, Ax, fx, Ay, fy) in f32 semantics."""
    rng = np.arange(-1, 2)
    pnx, pny = np.meshgrid(rng, rng, indexing="ij")
    pnx = pnx.reshape(-1).astype(np.float32)
    pny = pny.reshape(-1).astype(np.float32)
    out = {}
    for a in ANGLES:
        ox, oy = _OFF[a // 45]
        dx = pnx + np.array(ox, dtype=np.float32)
        dy = pny + np.array(oy, dtype=np.float32)
        rows = []
        for n in range(9):
            Ax = int(np.floor(dx[n]))
            Ay = int(np.floor(dy[n]))
            fx = float(np.float32(dx[n] - Ax))
            fy = float(np.float32(dy[n] - Ay))
            rows.append((n, n // 3, n % 3, Ax, fx, Ay, fy))
        out[a] = rows
    return out


TABLES = _tables()
# distinct fractional row offsets shared by the 45/135 pair
LERP_DS = sorted({(t[3], t[4]) for a in (45, 135) for t in TABLES[a]})

# Angle-0 phase-collapsed conv: output phase rho uses row taps di with the
# listed combo of original kernel rows (g(m)=m//3+m%3-1 collides for m=1,3
# and m=2,4). Combo indices into the host-precomputed sums: 0,1,2 = single
# ki, 3 = ki0+ki2. Same structure for columns. 49 taps/phase-grid vs 81.
ROW_COMBOS = [(0,), (1,), (2,), (0, 2)]
PHROWS = {0: [(-1, 0), (0, 1), (1, 2)],
          1: [(0, 3), (1, 1)],
          2: [(1, 3), (0, 1)]}


def build_graph():
    nc = bacc.Bacc()
    xs = nc.declare_dram_parameter("xs", [2, 2, 128, SLAB_ROWS * SLAB_COLS], F16, False)
    ww = nc.declare_dram_parameter("ww", [2, 128, 6 * 3 * 2 * 128], F16, False)
    w2 = nc.declare_dram_parameter("w2", [2, 128, 4 * 3 * 2 * 128], F16, False)
    wc = nc.declare_dram_parameter("wc", [2, 128, 16 * 2 * 128], F16, False)
    out = nc.declare_dram_parameter("out", [5, 2, 2, 128, NR, 190], F16, True)

    with tile.TileContext(nc) as tc:
        with (
            tc.tile_pool(name="const", bufs=1) as constp,
            tc.tile_pool(name="xop", bufs=2) as xop,
            tc.tile_pool(name="rcp", bufs=1) as rcp,
            tc.tile_pool(name="vp", bufs=2) as vp,
            tc.tile_pool(name="vtmp", bufs=1) as vtp,
            tc.tile_pool(name="mp", bufs=2) as mpp,
            tc.tile_pool(name="stg", bufs=2) as stgp,
            tc.tile_pool(name="ps", bufs=2, space="PSUM") as psp,
        ):
            # HAM warm-up: dependency-free matmuls on an uninitialized tile
            # keep the PE busy during the input-DMA window so the clock gate
            # is already at 8/8 when the first real matmul issues.
            warm = constp.tile([128, 384], F16, name="warm", tag="warm")
            nc.gpsimd.memset(warm[:], 0.0)
            wps = psp.tile([128, 1536], F32, name="wps", tag="ps")
            for _ in range(16):
                nc.tensor.matmul(wps[:, :256], warm[:, :128], warm[:, 128:384],
                                 start=True, stop=True)

            # DMA order matters for the head: the first job (collapsed
            # angle-0, batch 0) needs slab b0 + wc only; w2 is needed one
            # job later, ww three jobs later.
            slab = {}

            def load_slab(b):
                for ct in range(2):
                    s = constp.tile([128, SLAB_ROWS * SLAB_COLS], F16,
                                    name=f"slab{b}{ct}", tag=f"slab{b}{ct}")
                    nc.sync.dma_start(s[:], xs[b, ct])
                    slab[(b, ct)] = s

            load_slab(0)
            # wc is ot-major; load the ot=0 half first so the first job's
            # first matmuls only wait on half the collapsed-weight bytes.
            wc_sb = []
            for ct in range(2):
                wctile = constp.tile([128, 16 * 2 * 128], F16, name=f"wc{ct}",
                                     tag=f"wc{ct}")
                nc.sync.dma_start(wctile[:, :2048], wc[ct][:, :2048])
                wc_sb.append(wctile)
            for ct in range(2):
                nc.sync.dma_start(wc_sb[ct][:, 2048:], wc[ct][:, 2048:])
            w2_sb = []
            for ct in range(2):
                w2t = constp.tile([128, 4 * 3 * 2 * 128], F16, name=f"w2{ct}",
                                  tag=f"w2{ct}")
                nc.sync.dma_start(w2t[:], w2[ct])
                w2_sb.append(w2t)
            load_slab(1)
            ww_sb = []
            for ct in range(2):
                wwt = constp.tile([128, 6 * 3 * 2 * 128], F16, name=f"ww{ct}",
                                  tag=f"ww{ct}")
                nc.sync.dma_start(wwt[:], ww[ct])
                ww_sb.append(wwt)

            def slab3(b, ct):
                return slab[(b, ct)].rearrange("p (r c) -> p r c", c=SLAB_COLS)

            def build_xo_int(a, b):
                """xo tiles for an integer-offset angle via strided copies
                on the Scalar engine (Vector is the scarce resource)."""
                xo = []
                for ct in range(2):
                    t = xop.tile([128, XO2F], F16, name=f"xo{ct}", tag=f"xo{ct}")
                    v = t[:, :XO_F].rearrange("p (r c) -> p r c", c=192)
                    sv = slab3(b, ct)
                    for (n, r, s, Ax, fx, Ay, fy) in TABLES[a]:
                        nrow = 9 if r < 2 else 8
                        src = sv[:, 2 + Ax : 2 + Ax + nrow, 2 + Ay : 66 + Ay]
                        nc.scalar.copy(v[:, r::3, s::3], src)
                    xo.append(t)
                return xo

            # (Ax, fx) row-offsets that some fy!=0 tap reads: only these
            # need a col-diff C tile.
            needs_c = {(t[3], t[4]) for a in (45, 135) for t in TABLES[a]
                       if t[6] != 0.0}

            def build_lerp_rc(b):
                """Shared row-lerp R_d and col-diff C_d tiles for 45+135."""
                R = {}
                C = {}
                for ct in range(2):
                    sv = slab3(b, ct)
                    dr = rcp.tile([128, 12 * SLAB_COLS], F16,
                                  name=f"dr{ct}", tag="dr")
                    drv = dr.rearrange("p (r c) -> p r c", c=SLAB_COLS)
                    nc.vector.tensor_tensor(drv, sv[:, 1:13, :], sv[:, 0:12, :],
                                            AluOpType.subtract)
                    for di, (Ax, fx) in enumerate(LERP_DS):
                        if fx == 0.0:
                            rv = sv[:, 2 + Ax : 11 + Ax, :]
                        else:
                            rt = rcp.tile([128, 9 * SLAB_COLS], F16,
                                          name=f"r{ct}_{di}", tag=f"r{ct}_{di}")
                            rv = rt.rearrange("p (r c) -> p r c", c=SLAB_COLS)
                            nc.vector.scalar_tensor_tensor(
                                rv, drv[:, 2 + Ax : 11 + Ax, :], fx,
                                sv[:, 2 + Ax : 11 + Ax, :],
                                AluOpType.mult, AluOpType.add)
                        R[(ct, Ax, fx)] = rv
                        if (Ax, fx) in needs_c:
                            ctile = rcp.tile([128, 9 * SLAB_COLS], F16,
                                             name=f"c{ct}_{di}", tag=f"c{ct}_{di}")
                            cv = ctile.rearrange("p (r c) -> p r c", c=SLAB_COLS)
                            nc.vector.tensor_tensor(cv[:, :, 0:69], rv[:, :, 1:70],
                                                    rv[:, :, 0:69], AluOpType.subtract)
                            C[(ct, Ax, fx)] = cv
                return R, C

            def build_xo_lerp(a, b, R, C):
                xo = []
                for ct in range(2):
                    t = xop.tile([128, XO2F], F16, name=f"xo{ct}", tag=f"xo{ct}")
                    v = t[:, :XO_F].rearrange("p (r c) -> p r c", c=192)
                    for (n, r, s, Ax, fx, Ay, fy) in TABLES[a]:
                        nrow = 9 if r < 2 else 8
                        rv = R[(ct, Ax, fx)][:, :nrow, 2 + Ay : 66 + Ay]
                        if fy == 0.0:
                            nc.scalar.copy(v[:, r::3, s::3], rv)
                        else:
                            cv = C[(ct, Ax, fx)][:, :nrow, 2 + Ay : 66 + Ay]
                            nc.vector.scalar_tensor_tensor(
                                v[:, r::3, s::3], cv, fy, rv,
                                AluOpType.mult, AluOpType.add)
                    xo.append(t)
                return xo

            def conv_job_angle0(ai, b):
                """Phase-collapsed conv reading the slab directly (no xo).
                Groups by output row phase rho so each rho's rows can DMA
                out as soon as its three column phases are evacuated."""
                for ot in range(2):
                    for rho in range(3):
                        stg = stgp.tile([128, 8 * 192], F16, name="stg", tag="stg")
                        stgv = stg.rearrange("p (r c) -> p r c", c=192)
                        ps = psp.tile([128, 1536], F32, name="ps", tag="ps")
                        for sig in range(3):
                            taps = [(di, dj, ri * 4 + ci)
                                    for (di, ri) in PHROWS[rho]
                                    for (dj, ci) in PHROWS[sig]]
                            nmm = len(taps) * 2
                            i = 0
                            for (di, dj, cb) in taps:
                                for ct in range(2):
                                    sv = slab3(b, ct)
                                    w_ap = wc_sb[ct][:, (ot * 16 + cb) * 128 :
                                                     (ot * 16 + cb + 1) * 128]
                                    nc.tensor.matmul(
                                        ps[:, sig * 512 : (sig + 1) * 512],
                                        w_ap,
                                        sv[:, di + 2 : di + 10, dj + 2 : dj + 66],
                                        start=(i == 0), stop=(i == nmm - 1))
                                    i += 1
                        for sig in range(3):
                            psv = ps[:, sig * 512 : (sig + 1) * 512].rearrange(
                                "p (r c) -> p r c", c=64)
                            nc.scalar.copy(stgv[:, :, sig::3], psv)
                        nc.sync.dma_start(out[ai, b, ot, :, rho::3, :],
                                          stgv[:, :, :190])

            def wino_job_f4(ai, b, xo):
                """F(4,3) 1D row-Winograd conv of xo -> 24x190 outputs.
                6 groups of 4 output rows; per group 6 M_u chains of
                3(kj) x 2(ct) accumulating matmuls; AT combine on Vector.

                scalar_tensor_tensor runs 1x on the DVE, so every scaled
                combine is split into tensor_scalar (4x) + tensor_tensor
                (2x), and adjacent-row operand pairs are fused into one
                [128,3,2,192] op via a rows-of-4 rearrange of xo."""
                xov = [xo[ct][:, :XO_ROWS2 * 192].rearrange("p (r c) -> p r c", c=192)
                       for ct in range(2)]
                for half in range(2):
                    g0 = 3 * half

                    def dpairc(ct, a):
                        """rows (4g+a, 4g+a+1) for the half's 3 groups:
                        [128, 3, 2, 192]."""
                        a0 = 4 * g0 + a
                        v4 = xov[ct][:, a0 : a0 + 12, :].rearrange(
                            "p (g r) c -> p g r c", r=4)
                        return v4[:, :, 0:2, :]

                    def dpair2(ct, a):
                        """rows (4g+a, 4g+a+2): [128, 3, 2, 192]."""
                        a0 = 4 * g0 + a
                        v4 = xov[ct][:, a0 : a0 + 12, :].rearrange(
                            "p (g r) c -> p g r c", r=4)
                        return v4[:, :, 0:3:2, :]

                    vt = []
                    for ct in range(2):
                        t = vp.tile([128, 3 * 6 * 192], F16, name=f"v{ct}",
                                    tag=f"v{ct}")
                        v = t.rearrange("p (g u c) -> p g u c", u=6, c=192)
                        tmp = vtp.tile([128, 3 * 8 * 192], F16, name=f"vt{ct}",
                                       tag=f"vt{ct}")
                        tm = tmp.rearrange("p (g i c) -> p g i c", i=8, c=192)
                        pairc = lambda i: tm[:, :, i : i + 2, :]
                        # ab = -5*(d2,d3); tAtB = ab + (d4,d5); ce = 4*(d0,d1)
                        nc.vector.tensor_scalar_mul(pairc(0), dpairc(ct, 2), -5.0)
                        nc.vector.tensor_tensor(pairc(2), pairc(0), dpairc(ct, 4),
                                                AluOpType.add)
                        nc.vector.tensor_scalar_mul(pairc(4), dpairc(ct, 0), 4.0)
                        # (v0, v5) = ce + tAtB
                        nc.vector.tensor_tensor(v[:, :, 0:6:5, :],
                                                tm[:, :, 4:6, :], tm[:, :, 2:4, :],
                                                AluOpType.add)
                        # s = (d1,d3)+(d2,d4); m = (d1,d3)-(d2,d4)  [GpSimd]
                        nc.gpsimd.tensor_tensor(tm[:, :, 6:8, :], dpair2(ct, 1),
                                                dpair2(ct, 2), AluOpType.add)
                        nc.gpsimd.tensor_tensor(tm[:, :, 0:2, :], dpair2(ct, 1),
                                                dpair2(ct, 2), AluOpType.subtract)
                        # v1 = -4*s1 + s2 ; v2 = 4*m1 - m2
                        nc.vector.tensor_scalar_mul(tm[:, :, 2], tm[:, :, 6], -4.0)
                        nc.vector.tensor_tensor(v[:, :, 1], tm[:, :, 2], tm[:, :, 7],
                                                AluOpType.add)
                        nc.vector.tensor_scalar_mul(tm[:, :, 3], tm[:, :, 0], 4.0)
                        nc.vector.tensor_tensor(v[:, :, 2], tm[:, :, 3], tm[:, :, 1],
                                                AluOpType.subtract)
                        # p = (d3,d4)-(d1,d2); v3 = 2*p1+p2; v4 = p2-2*p1
                        nc.vector.tensor_tensor(pairc(4), dpairc(ct, 3), dpairc(ct, 1),
                                                AluOpType.subtract)
                        nc.vector.tensor_scalar_mul(tm[:, :, 6], tm[:, :, 4], 2.0)
                        nc.vector.tensor_tensor(v[:, :, 3], tm[:, :, 6], tm[:, :, 5],
                                                AluOpType.add)
                        nc.vector.tensor_tensor(v[:, :, 4], tm[:, :, 5], tm[:, :, 6],
                                                AluOpType.subtract)
                        vt.append(v)
                    for ot in range(2):
                        mt = mpp.tile([128, 3 * 6 * 192], F16, name="m", tag="m")
                        mv = mt.rearrange("p (g u c) -> p g u c", u=6, c=192)
                        for gi in range(3):
                            ps = psp.tile([128, 1536], F32, name="ps", tag="ps")
                            for u in range(6):
                                i = 0
                                for kj in range(3):
                                    for ct in range(2):
                                        w_ap = ww_sb[ct][:, ((u * 3 + kj) * 2 + ot) * 128 :
                                                         ((u * 3 + kj) * 2 + ot + 1) * 128]
                                        nc.tensor.matmul(
                                            ps[:, u * 256 : u * 256 + 192 - kj],
                                            w_ap,
                                            vt[ct][:, gi, u, kj:192],
                                            start=(i == 0), stop=(i == 5))
                                        i += 1
                            psv = ps.rearrange("p (u c) -> p u c", c=256)
                            nc.scalar.copy(mv[:, gi], psv[:, :6, :192])
                        # AT combine: y = AT @ M for the half's 3 groups.
                        # yt slots: 0:s12 1:s34 2:d12 3:d34 4:t0 5:q1 6:q2 7:q3
                        ytmp = vtp.tile([128, 3 * 8 * 192], F16, name="yt", tag="yt")
                        yt = ytmp.rearrange("p (g i c) -> p g i c", i=8, c=192)
                        stg = stgp.tile([128, 12 * 192], F16, name="stg", tag="stg")
                        sg4 = stg.rearrange("p (g r c) -> p g r c", r=4, c=192)
                        yrow = lambda i: sg4[:, :, i, :]
                        M = lambda u: mv[:, :, u, :]
                        # (s12,s34) = (M1,M3)+(M2,M4); (d12,d34) = (M1,M3)-(M2,M4)
                        nc.vector.tensor_tensor(yt[:, :, 0:2, :], mv[:, :, 1:4:2, :],
                                                mv[:, :, 2:5:2, :], AluOpType.add)
                        nc.vector.tensor_tensor(yt[:, :, 2:4, :], mv[:, :, 1:4:2, :],
                                                mv[:, :, 2:5:2, :], AluOpType.subtract)
                        nc.vector.tensor_tensor(yt[:, :, 4], M(0), yt[:, :, 0],
                                                AluOpType.add)                      # t0
                        nc.vector.tensor_scalar_mul(yt[:, :, 5], yt[:, :, 3], 2.0)  # q1
                        nc.vector.tensor_scalar_mul(yt[:, :, 6], yt[:, :, 1], 4.0)  # q2
                        nc.vector.tensor_scalar_mul(yt[:, :, 7], yt[:, :, 3], 8.0)  # q3
                        # (y0,y1) = (t0,q1) + (s34,d12)
                        nc.vector.tensor_tensor(sg4[:, :, 0:2, :], yt[:, :, 4:6, :],
                                                yt[:, :, 1:3, :], AluOpType.add)
                        nc.vector.tensor_tensor(yrow(2), yt[:, :, 6], yt[:, :, 0],
                                                AluOpType.add)                      # y2
                        nc.vector.tensor_tensor(yt[:, :, 4], yt[:, :, 7], yt[:, :, 2],
                                                AluOpType.add)                      # y3t
                        nc.vector.tensor_tensor(yrow(3), yt[:, :, 4], M(5),
                                                AluOpType.add)                      # y3
                        sg = stg.rearrange("p (r c) -> p r c", c=192)
                        nc.sync.dma_start(
                            out[ai, b, ot, :, 12 * half : 12 * half + 12, :],
                            sg[:, :, :190])

            def wino_job_f2(ai, b, xo):
                """F(2,3) 1D row-Winograd conv of xo -> 24x190 outputs.
                12 pairs of output rows, processed 2 pairs per chunk (one
                PSUM tile = 8 M_u chains, one evac copy)."""
                xov = [xo[ct][:, :XO_F].rearrange("p (r c) -> p r c", c=192)
                       for ct in range(2)]
                for half in range(2):
                    stg = [stgp.tile([128, 12 * 192], F16, name="stg", tag="stg")
                           for _ in range(2)]
                    sgp = [s.rearrange("p (q r c) -> p q r c", r=2, c=192)
                           for s in stg]
                    for c3 in range(3):
                        p0 = 6 * half + 2 * c3
                        vt = []
                        for ct in range(2):
                            t = vp.tile([128, 2 * 4 * 192], F16, name=f"v{ct}",
                                        tag=f"v{ct}")
                            v = t.rearrange("p (g u c) -> p g u c", u=4, c=192)
                            d = [xov[ct][:, 2 * p0 + a : 2 * p0 + a + 3 : 2, :]
                                 for a in range(4)]
                            # (u0,u3) = (d0,d1)-(d2,d3) in one op; u2 is
                            # built sign-flipped (d1-d2) with the weight row
                            # negated on the host.
                            rp = lambda a: xov[ct][:, 2 * p0 + a : 2 * p0 + a + 4, :
                                                   ].rearrange("p (g r) c -> p g r c", r=2)
                            nc.vector.tensor_tensor(v[:, :, 0:4:3, :], rp(0), rp(2),
                                                    AluOpType.subtract)
                            nc.vector.tensor_tensor(v[:, :, 1], d[1], d[2], AluOpType.add)
                            nc.vector.tensor_tensor(v[:, :, 2], d[1], d[2], AluOpType.subtract)
                            vt.append(v)
                        for ot in range(2):
                            mt = mpp.tile([128, 2 * 4 * 192], F16, name="m", tag="m")
                            mv = mt.rearrange("p (g u c) -> p g u c", u=4, c=192)
                            ps = psp.tile([128, 2048], F32, name="ps", tag="ps")
                            for p2 in range(2):
                                for u in range(4):
                                    # slot m=p2*4+u at bank (m//2), half (m%2)
                                    m = p2 * 4 + u
                                    off = (m // 2) * 512 + (m % 2) * 192
                                    i = 0
                                    for kj in range(3):
                                        for ct in range(2):
                                            w_ap = w2_sb[ct][:, ((u * 3 + kj) * 2 + ot) * 128 :
                                                             ((u * 3 + kj) * 2 + ot + 1) * 128]
                                            nc.tensor.matmul(
                                                ps[:, off : off + 192 - kj],
                                                w_ap,
                                                vt[ct][:, p2, u, kj:192],
                                                start=(i == 0), stop=(i == 5))
                                            i += 1
                            psv = ps.rearrange("p (q x) -> p q x", x=512)[:, :, :384]
                            psv = psv.rearrange("p q (m c) -> p q m c", c=192)
                            mvd = mt.rearrange("p (q m c) -> p q m c", q=4, c=192)
                            nc.scalar.copy(mvd, psv)
                            ytmp = vtp.tile([128, 2 * 2 * 192], F16, name="y2", tag="yt")
                            yt = ytmp.rearrange("p (i g c) -> p i g c", g=2, c=192)
                            yrow = lambda i: sgp[ot][:, 2 * c3 : 2 * c3 + 2, i, :]
                            M = lambda u: mv[:, :, u, :]
                            # u2's V-row AND its G row are both negated, so
                            # M2 keeps its textbook sign.
                            nc.vector.tensor_tensor(yt[:, 0], M(1), M(2), AluOpType.add)
                            nc.vector.tensor_tensor(yrow(0), M(0), yt[:, 0], AluOpType.add)
                            nc.vector.tensor_tensor(yt[:, 1], M(1), M(2), AluOpType.subtract)
                            nc.vector.tensor_tensor(yrow(1), yt[:, 1], M(3), AluOpType.subtract)
                    for ot in range(2):
                        sg = stg[ot].rearrange("p (r c) -> p r c", c=192)
                        nc.sync.dma_start(
                            out[ai, b, ot, :, 12 * half : 12 * half + 12, :],
                            sg[:, :, :190])

            # angle 0 (no xo build) first for b=0 so the PE starts on the
            # slab DMA alone, and last for b=1 so the tail is the staggered
            # per-rho DMAs of the collapsed job.
            conv_job_angle0(0, 0)
            for b in range(2):
                R, C = build_lerp_rc(b)
                xo = build_xo_lerp(45, b, R, C)
                wino_job_f2(1, b, xo)
                xo = build_xo_lerp(135, b, R, C)
                wino_job_f2(3, b, xo)
                xo = build_xo_int(90, b)
                wino_job_f4(2, b, xo)
                xo = build_xo_int(180, b)
                wino_job_f4(4, b, xo)
            conv_job_angle0(0, 1)

    nc.compile()
    return nc


_GRAPH = None


def _graph():
    global _GRAPH
    if _GRAPH is None:
        _GRAPH = build_graph()
    return _GRAPH


def prep_inputs(x, weight):
    x = np.asarray(x, dtype=np.float32)
    weight = np.asarray(weight, dtype=np.float32)
    # pad data rows -2..66, cols -2..67
    xp = np.pad(x, ((0, 0), (0, 0), (2, 3), (2, 4))).astype(np.float16)
    xs_cores = []
    for k in range(NCORES):
        sl = xp[:, :, 8 * k : 8 * k + SLAB_ROWS, :]          # [2,256,13,70]
        sl = sl.reshape(2, 2, 128, SLAB_ROWS * SLAB_COLS)
        xs_cores.append(np.ascontiguousarray(sl))
    w6 = weight.reshape(2, 128, 2, 128, 3, 3)                 # [ot,o,ct,c,ki,kj]

    def wino_w(G):
        nu = G.shape[0]
        wt = np.zeros((nu, 2, 128, 2, 128, 3), np.float32)    # [u,ot,o,ct,c,kj]
        for u in range(nu):
            for ki in range(3):
                wt[u] += G[u, ki] * w6[:, :, :, :, ki, :]
        # -> [ct, c, u, kj, ot, o] -> [2, 128, nu*3*2*128]
        wt = wt.transpose(3, 4, 0, 5, 1, 2).reshape(2, 128, nu * 3 * 2 * 128)
        return np.ascontiguousarray(wt.astype(np.float16))

    wwarr = wino_w(G43)
    w2arr = wino_w(G23)

    combos = []
    for Rr in ROW_COMBOS:
        for Cc in ROW_COMBOS:
            combos.append(w6[..., list(Rr), :][..., list(Cc)].sum(axis=(-1, -2)))
    wcarr = np.stack(combos, axis=0)                          # [16,ot,o,ct,c]
    wcarr = wcarr.transpose(3, 4, 1, 0, 2).reshape(2, 128, 2 * 16 * 128)
    wcarr = np.ascontiguousarray(wcarr.astype(np.float16))
    return xs_cores, wwarr, w2arr, wcarr


def assemble(results):
    full = np.empty((5, 2, 256, NCORES * NR, 190), np.float32)
    for k in range(NCORES):
        o = results[k]["out"].astype(np.float32)              # [5,2,2,128,24,190]
        o = o.reshape(5, 2, 256, NR, 190)
        full[:, :, :, NR * k : NR * (k + 1), :] = o
    full = full[:, :, :, :190, :]
    return tuple(np.ascontiguousarray(full[i]) for i in range(5))


def run(x, weight, trace=False, **trace_kw):
    xs_cores, wwarr, w2arr, wcarr = prep_inputs(x, weight)
    nc = _graph()
    in_maps = [{"xs": xs_cores[k], "ww": wwarr, "w2": w2arr, "wc": wcarr}
               for k in range(NCORES)]
    res = run_bass_kernel_spmd(nc, in_maps, core_ids=list(range(NCORES)),
                               trace=trace, **trace_kw)
    return assemble(res.results), res


def kernel(x, weight):
    return run(x, weight)[0]
